# revision 14
# baseline (speedup 1.0000x reference)
"""DeepIRT (DKVMN) Trainium2 kernel.

Contract: kernel(**inputs) takes the FULL unsharded inputs of reference.py's
setup_inputs() and returns the full [64, 500] float32 output.

Strategy (8 NeuronCores, pure data parallel over batch):
  - each core handles BL=8 batch rows; tables/weights replicated.
  - precompute per core: gather k/v embeddings (indirect DMA), transpose to
    [d, token] layout with PE, compute w = softmax(k Mk^T), e = sigmoid(v We+be),
    a = tanh(v Wa+ba); pack per-step PE coefficient streams into DRAM:
      wd9[t]  = [9, 400]: row 0 = ones(400); row 1+b = w[b,t,:] placed in a
                block-diagonal at columns [b*50, b*50+50)
      lg9[t]  = [9, 128]: row 0 = ones(128); row 1+b = -e[b,t,:]
      la9[t]  = [9, 128]: row 0 = zeros;     row 1+b =  a[b,t,:]
  - recurrence over t (state Mv layout [128(d), 8b*50m] in SBUF):
      G  = lg9[t]^T @ wd9[t]  -> PSUM [128, 400] = 1 - w (x) e   (PE)
      WA = la9[t]^T @ wd9[t]  -> PSUM [128, 400] =      w (x) a  (PE)
      X  = Mv * G                                   (DVE tensor_tensor)
      Mv = X + WA                                   (DVE tensor_tensor)
      RX = reduce_m(X)          [128, 8]            (DVE tensor_reduce)
      read_t = (S_prev - RX) * (1/e_t)  (since reduce_m(Mv*(w e)) = e * read)
      S      = RX + a_t                 (since softmax rows sum to 1)
    1/e is computed exactly as 1 + exp(-z) from the sigmoid logits z.
  - final: f = tanh([reads, k] Wf + bf) via two accumulating matmuls on the
    [d, token] archives; stu/qd heads via [128,1] matmuls into an [8, 500]
    PSUM tile; predict = sigmoid(3*stu - qd) -> output [8, 500].

Tokens are ordered b-major: tok = b*T_PAD + t with T padded to T_PAD=512
(4096 tokens) so every 128-token tile is full and lies within one batch row;
padded slots use index 0 and are never read by the recurrence or the output
path.
"""

import numpy as np

import concourse.bass as bass
import concourse.bacc as bacc
import concourse.tile as tile
import concourse.mybir as mybir
from concourse.masks import make_identity

F32 = mybir.dt.float32
BF16 = mybir.dt.bfloat16
I32 = mybir.dt.int32
I16 = mybir.dt.int16
OP = mybir.AluOpType
AF = mybir.ActivationFunctionType

NUM_CONCEPT = 1000
D = 128
M = 50
B_FULL, T = 64, 500
NCORES = 8
BL = B_FULL // NCORES          # 8 batch rows per core
T_PAD = 512
NTOK = T_PAD * BL              # 4096 padded tokens, b-major: tok = b*T_PAD + t
NCH = NTOK // 128              # 32 gather/transpose chunks
W9 = 9 * 400                   # wd9 row stride in elements
L9 = 9 * 128
A9 = 9 * 128
STEP_CHUNK = 16                # recurrence steps loaded per DMA


def _ap(t, offset, dims):
    return bass.AP(t.tensor, offset, [list(d) for d in dims])


def _evac(nc, i, out, in_):
    """PSUM->SBUF copy, round-robined over DVE / ACT (Pool can't read PSUM)."""
    if i % 2 == 0:
        nc.vector.tensor_copy(out=out, in_=in_)
    else:
        nc.scalar.copy(out=out, in_=in_)


def build_program(debug_taps=False, phases=(1, 2, 3, 4)):
    nc = bacc.Bacc("TRN2", target_bir_lowering=False, debug=False)

    # ---------------- I/O ----------------
    h = {}
    h["concept_seq"] = nc.declare_dram_parameter("concept_seq", [BL, T], I32, isOutput=False)
    h["correct_seq"] = nc.declare_dram_parameter("correct_seq", [BL, T], I32, isOutput=False)
    h["embed_key"] = nc.declare_dram_parameter("embed_key", [NUM_CONCEPT, D], F32, isOutput=False)
    h["embed_value"] = nc.declare_dram_parameter("embed_value", [2 * NUM_CONCEPT, D], F32, isOutput=False)
    h["Mk"] = nc.declare_dram_parameter("Mk", [M, D], F32, isOutput=False)
    h["Mv0"] = nc.declare_dram_parameter("Mv0", [M, D], F32, isOutput=False)
    h["We"] = nc.declare_dram_parameter("We", [D, D], F32, isOutput=False)
    h["be"] = nc.declare_dram_parameter("be", [D], F32, isOutput=False)
    h["Wa"] = nc.declare_dram_parameter("Wa", [D, D], F32, isOutput=False)
    h["ba"] = nc.declare_dram_parameter("ba", [D], F32, isOutput=False)
    h["Wf"] = nc.declare_dram_parameter("Wf", [2 * D, D], F32, isOutput=False)
    h["bf"] = nc.declare_dram_parameter("bf", [D], F32, isOutput=False)
    h["Wab"] = nc.declare_dram_parameter("Wab", [D, 1], F32, isOutput=False)
    h["bab"] = nc.declare_dram_parameter("bab", [1], F32, isOutput=False)
    h["Wd"] = nc.declare_dram_parameter("Wd", [D, 1], F32, isOutput=False)
    h["bd"] = nc.declare_dram_parameter("bd", [1], F32, isOutput=False)
    out_h = nc.declare_dram_parameter("out", [BL, T], F32, isOutput=True)
    dbg = {}
    if debug_taps:
        for n in ("dbg_eT", "dbg_aT", "dbg_erecip", "dbg_fT"):
            dbg[n] = nc.declare_dram_parameter(n, [128, NTOK], F32, isOutput=True)
        for n in ("dbg_kT", "dbg_reads"):
            dbg[n] = nc.declare_dram_parameter(n, [128, NTOK], BF16, isOutput=True)
        dbg["dbg_state"] = nc.declare_dram_parameter("dbg_state", [128, BL * M], F32, isOutput=True)
        dbg["dbg_prob"] = nc.declare_dram_parameter("dbg_prob", [1, NTOK], F32, isOutput=True)
        dbg["dbg_wd9"] = nc.declare_dram_parameter("dbg_wd9", [T_PAD, 9, 400], BF16, isOutput=True)
        dbg["dbg_lg9"] = nc.declare_dram_parameter("dbg_lg9", [T_PAD, 9, 128], BF16, isOutput=True)
        dbg["dbg_la9"] = nc.declare_dram_parameter("dbg_la9", [T_PAD, 9, 128], BF16, isOutput=True)

    with tile.TileContext(nc) as tc:
        _emit(nc, tc, h, out_h, dbg, phases)
    nc.finalize()
    return nc


def _emit(nc, tc, h, out_h, dbg=None, phases=(1, 2, 3, 4)):
    from contextlib import ExitStack

    ctx = ExitStack()
    with ctx:
        # ---- pools ----
        persist = ctx.enter_context(tc.tile_pool(name="persist", bufs=1))
        dram = ctx.enter_context(tc.tile_pool(name="dram", bufs=1, space="DRAM"))

        # DRAM coefficient streams
        wd9 = dram.tile([T_PAD, 9, 400], BF16)
        lg9 = dram.tile([T_PAD, 9, 128], BF16)
        la9 = dram.tile([T_PAD, 9, 128], BF16)

        # persistent SBUF archives ([d, token] layouts, t-major tokens)
        k_T = persist.tile([128, NTOK], BF16)
        v_T = persist.tile([128, NTOK], BF16)
        e_T = persist.tile([128, NTOK], BF16)
        erecip_T = persist.tile([128, NTOK], F32)
        a_T = persist.tile([128, NTOK], BF16)
        reads_T = persist.tile([128, NTOK], BF16)
        f_T = persist.tile([128, NTOK], F32)

        # small persistent constants / weights
        ident = persist.tile([128, 128], F32)
        make_identity(nc, ident)
        ident_bf = persist.tile([128, 128], BF16)
        nc.vector.tensor_copy(out=ident_bf, in_=ident)
        ones128 = persist.tile([128, 128], BF16)
        nc.vector.memset(ones128, 1.0)
        ones400 = persist.tile([128, 400], BF16)
        nc.vector.memset(ones400, 1.0)
        zeros1200 = persist.tile([128, 1200], BF16)
        nc.vector.memset(zeros1200, 0.0)
        zeros400f = persist.tile([1, 400], F32)
        nc.vector.memset(zeros400f, 0.0)

        We_f32 = persist.tile([128, 128], F32)
        nc.sync.dma_start(out=We_f32, in_=h["We"][:, :])
        We_sb = persist.tile([128, 128], BF16)
        nc.vector.tensor_copy(out=We_sb, in_=We_f32)
        Wa_f32 = persist.tile([128, 128], F32)
        nc.sync.dma_start(out=Wa_f32, in_=h["Wa"][:, :])
        Wa_sb = persist.tile([128, 128], BF16)
        nc.vector.tensor_copy(out=Wa_sb, in_=Wa_f32)
        Wf_r32 = persist.tile([128, 128], F32)
        nc.sync.dma_start(out=Wf_r32, in_=h["Wf"][0:128, :])
        Wf_r = persist.tile([128, 128], BF16)
        nc.vector.tensor_copy(out=Wf_r, in_=Wf_r32)
        Wf_k32 = persist.tile([128, 128], F32)
        nc.sync.dma_start(out=Wf_k32, in_=h["Wf"][128:256, :])
        Wf_k = persist.tile([128, 128], BF16)
        nc.vector.tensor_copy(out=Wf_k, in_=Wf_k32)
        Wab_sb = persist.tile([128, 1], F32)
        nc.sync.dma_start(out=Wab_sb, in_=h["Wab"][:, :])
        Wd_sb = persist.tile([128, 1], F32)
        nc.sync.dma_start(out=Wd_sb, in_=h["Wd"][:, :])
        Mk_sb = persist.tile([50, 128], F32)
        nc.sync.dma_start(out=Mk_sb, in_=h["Mk"][:, :])
        Mv0_sb = persist.tile([50, 128], F32)
        nc.sync.dma_start(out=Mv0_sb, in_=h["Mv0"][:, :])

        def col(name, n=128):
            t = persist.tile([n, 1], F32, name=name)
            nc.sync.dma_start(out=t, in_=_ap(h[name[:-4]][:], 0, [[1, n], [1, 1]]))
            return t

        be_col = col("be_col")
        ba_col = col("ba_col")
        bf_col = col("bf_col")
        neg_be_col = persist.tile([128, 1], F32)
        nc.vector.tensor_scalar(out=neg_be_col, in0=be_col, scalar1=-1.0, scalar2=None, op0=OP.mult)

        # ---- stream-layout views ----
        wd9f = wd9.rearrange("t r c -> t (r c)")
        lg9f = lg9.rearrange("t r c -> t (r c)")
        la9f = la9.rearrange("t r c -> t (r c)")

        # ---- transpose Mv0 and Mk once; Mv0^T kept in SBUF for state init ----
        Mv0T_sb = persist.tile([128, 50], F32)
        MkT_sb = persist.tile([128, 50], BF16)
        with tc.tile_pool(name="init_ps", bufs=1, space="PSUM") as initp:
            mv0t = initp.tile([128, 50], F32)
            nc.tensor.transpose(mv0t, Mv0_sb, ident[0:50, 0:50])
            nc.any.tensor_copy(out=Mv0T_sb, in_=mv0t)
            mkt_ps = initp.tile([128, 50], F32)
            nc.tensor.transpose(mkt_ps, Mk_sb, ident[0:50, 0:50])
            nc.any.tensor_copy(out=MkT_sb, in_=mkt_ps)

        # =========== phases 1+2 interleaved ===========
        if 1 not in phases or 2 not in phases:
            return
        idxk_dram = dram.tile([NTOK], I32)
        idxv_dram = dram.tile([NTOK], I32)
        ek_bf = dram.tile([NUM_CONCEPT, 128], BF16)
        ev_bf = dram.tile([2 * NUM_CONCEPT, 128], BF16)
        with tc.tile_pool(name="ph1", bufs=1) as ph1, \
             tc.tile_pool(name="ph1t", bufs=4) as ph1t, \
             tc.tile_pool(name="ph1ps", bufs=2, space="PSUM") as ph1ps, \
             tc.tile_pool(name="ph2", bufs=3) as ph2, \
             tc.tile_pool(name="ph2ps", bufs=1, space="PSUM") as ph2ps:

            # ---- index prep FIRST so the gather queue starts early ----
            cseq = ph1.tile([8, T_PAD], I32)
            crse = ph1.tile([8, T_PAD], I32)
            nc.vector.memset(cseq, 0)
            nc.vector.memset(crse, 0)
            nc.sync.dma_start(out=cseq[:, 0:T], in_=h["concept_seq"][:, :])
            nc.scalar.dma_start(out=crse[:, 0:T], in_=h["correct_seq"][:, :])

            # x = concept + 1000*correct (exact in fp32, cast back to ints)
            cseq_f = ph1.tile([8, T_PAD], F32)
            nc.vector.tensor_copy(out=cseq_f, in_=cseq)
            crse_f = ph1.tile([8, T_PAD], F32)
            nc.vector.tensor_copy(out=crse_f, in_=crse)
            x_f = ph1.tile([8, T_PAD], F32)
            nc.vector.scalar_tensor_tensor(out=x_f, in0=crse_f, scalar=float(NUM_CONCEPT),
                                           in1=cseq_f, op0=OP.mult, op1=OP.add)
            x_i = ph1.tile([8, T_PAD], I32)
            nc.vector.tensor_copy(out=x_i, in_=x_f)

            # bounce through DRAM to rewrap indices token-major: chunk g's
            # 128 tokens land as column g of a [128, NCH] tile
            ckw = ph1.tile([128, NCH], I32)
            xvw = ph1.tile([128, NCH], I32)
            for srct, drt, dstt, eng in ((x_i, idxv_dram, xvw, nc.scalar),
                                         (cseq, idxk_dram, ckw, nc.sync)):
                eng.dma_start(out=_ap(drt[:], 0, [[T_PAD, 8], [1, T_PAD]]), in_=srct)
                eng.dma_start(out=dstt, in_=_ap(drt[:], 0, [[1, 128], [128, NCH]]))

            # bf16 copies of the embedding tables in DRAM (halves gather
            # bytes); value table first so v-gathers start earliest
            for src_h, dst_t, nrows, base in ((h["embed_value"], ev_bf, 2 * NUM_CONCEPT, 0),
                                              (h["embed_key"], ek_bf, NUM_CONCEPT, 16)):
                for r in range(nrows // 128):
                    tl = ph1t.tile([128, 128], F32, tag="tcv")
                    eng = (nc.sync, nc.scalar)[r % 2]
                    eng.dma_start(out=tl, in_=src_h[r * 128:(r + 1) * 128, :])
                    tb = ph1t.tile([128, 128], BF16, tag="tcb")
                    nc.vector.tensor_copy(out=tb, in_=tl)
                    eng.dma_start(out=dst_t[r * 128:(r + 1) * 128, :], in_=tb)
                for r in range(nrows // 128, (nrows + 127) // 128):
                    n = nrows - 128 * (nrows // 128)
                    if n:
                        tl = ph1t.tile([128, 128], F32, tag="tcv")
                        nc.sync.dma_start(out=tl[0:n, :], in_=src_h[r * 128:r * 128 + n, :])
                        tb = ph1t.tile([128, 128], BF16, tag="tcb")
                        nc.vector.tensor_copy(out=tb[0:n, :], in_=tl[0:n, :])
                        nc.sync.dma_start(out=dst_t[r * 128:r * 128 + n, :], in_=tb[0:n, :])

            # ---- fill DRAM streams (sync/scalar only; gpsimd queue is for
            # gathers) ----
            fillqs = (nc.sync, nc.scalar)
            fq = 0
            for r0 in range(0, T_PAD, 128):
                for c0 in range(0, 3600, 1200):
                    fillqs[fq % 2].dma_start(out=wd9f[r0:r0 + 128, c0:c0 + 1200], in_=zeros1200)
                    fq += 1
                fillqs[fq % 2].dma_start(out=wd9[r0:r0 + 128, 0, :], in_=ones400); fq += 1
                fillqs[fq % 2].dma_start(out=lg9[r0:r0 + 128, 0, :], in_=ones128); fq += 1
                # lg rows 1-8 written by e pass; zero them anyway for padded tail rows
                fillqs[fq % 2].dma_start(out=lg9f[r0:r0 + 128, 128:1152],
                                         in_=zeros1200[:, 0:1024]); fq += 1
                fillqs[fq % 2].dma_start(out=la9f[r0:r0 + 128, 0:1152],
                                         in_=zeros1200[:, 0:1152]); fq += 1

            def gather_chunk(g, table, idx_tile, dst, tag):
                rows = ph1t.tile([128, 128], BF16, tag=tag)
                nc.gpsimd.indirect_dma_start(
                    out=rows, out_offset=None, in_=table,
                    in_offset=bass.IndirectOffsetOnAxis(ap=idx_tile[:, g:g + 1], axis=0))
                tps = ph1ps.tile([128, 128], BF16, tag="gt")
                nc.tensor.transpose(tps, rows, ident_bf)
                _evac(nc, g, dst[:, 128 * g:128 * (g + 1)], tps)

            # ---- loop A: v-gathers interleaved with e/a slice compute ----
            for c in range(8):
                for g in range(4 * c, 4 * c + 4):
                    gather_chunk(g, ev_bf[:, :], xvw, v_T, "rv")
                sl = slice(c * 512, (c + 1) * 512)
                elog = ph2ps.tile([128, 512], F32, tag="ealog", bufs=2)
                nc.tensor.matmul(elog, We_sb, v_T[:, sl], start=True, stop=True)
                nc.scalar.activation(out=e_T[:, sl], in_=elog, func=AF.Sigmoid, bias=be_col)
                etmp = ph2.tile([128, 512], F32, tag="etmp")
                nc.scalar.activation(out=etmp, in_=elog, func=AF.Exp,
                                     bias=neg_be_col, scale=-1.0)
                nc.vector.tensor_scalar(out=erecip_T[:, sl], in0=etmp, scalar1=1.0,
                                        scalar2=None, op0=OP.add)
                alog = ph2ps.tile([128, 512], F32, tag="ealog", bufs=2)
                nc.tensor.matmul(alog, Wa_sb, v_T[:, sl], start=True, stop=True)
                nc.scalar.activation(out=a_T[:, sl], in_=alog, func=AF.Tanh, bias=ba_col)

            # ---- loop B: k-gathers interleaved with w softmax + lg/la rows,
            # t0-major so early recurrence steps are scattered first ----
            for i in range(NCH):
                t0i, b = i // 8, i % 8
                p = 4 * b + t0i
                gather_chunk(p, ek_bf[:, :], ckw, k_T, "rk")
                sl = slice(p * 128, (p + 1) * 128)
                wlog = ph2ps.tile([128, 50], F32, tag="wlog", bufs=1)
                nc.tensor.matmul(wlog, k_T[:, sl], MkT_sb, start=True, stop=True)
                negmax = ph2.tile([128, 1], F32, tag="negmax")
                nc.vector.tensor_reduce(out=negmax, in_=wlog, axis=mybir.AxisListType.X,
                                        op=OP.max, negate=True)
                wexp = ph2.tile([128, 50], F32, tag="wexp")
                sumexp = ph2.tile([128, 1], F32, tag="sumexp")
                nc.scalar.activation(out=wexp, in_=wlog, func=AF.Exp, bias=negmax,
                                     accum_out=sumexp)
                rsum = ph2.tile([128, 1], F32, tag="rsum")
                nc.vector.reciprocal(out=rsum, in_=sumexp)
                w_sb = ph2.tile([128, 50], BF16, tag="w_sb")
                nc.vector.tensor_scalar(out=w_sb, in0=wexp, scalar1=rsum, scalar2=None,
                                        op0=OP.mult)
                # scatter into wd9[t0+q, 1+b, b*50 : b*50+50] (chunk p: b = p//4)
                t0 = 128 * t0i
                qs = (nc.sync, nc.scalar, nc.gpsimd)
                qs[p % 3].dma_start(
                    out=_ap(wd9f[:], t0 * W9 + 400 + 450 * b, [[W9, 128], [1, 50]]),
                    in_=w_sb)

                # e rows -> lg9 rows 1..8 (negated); a rows -> la9 rows 1..8
                # evacs alternate DVE/ACT so neither engine serializes phase 2
                ert = ph2ps.tile([128, 128], BF16, tag="eat", bufs=2)
                nc.tensor.transpose(ert, e_T[:, sl], ident_bf)
                erow = ph2.tile([128, 128], BF16, tag="erow")
                if p % 2 == 0:
                    nc.vector.tensor_scalar(out=erow, in0=ert, scalar1=-1.0,
                                            scalar2=None, op0=OP.mult)
                else:
                    nc.scalar.mul(out=erow, in_=ert, mul=-1.0)
                qs[(p + 1) % 3].dma_start(
                    out=_ap(lg9f[:], t0 * L9 + 128 * (1 + b), [[L9, 128], [1, 128]]),
                    in_=erow)
                art = ph2ps.tile([128, 128], BF16, tag="eat", bufs=2)
                nc.tensor.transpose(art, a_T[:, sl], ident_bf)
                arow = ph2.tile([128, 128], BF16, tag="arow")
                if p % 2 == 0:
                    nc.scalar.copy(out=arow, in_=art)
                else:
                    nc.vector.tensor_copy(out=arow, in_=art)
                qs[(p + 2) % 3].dma_start(
                    out=_ap(la9f[:], t0 * A9 + 128 * (1 + b), [[A9, 128], [1, 128]]),
                    in_=arow)

        # =========== phase 3: the 500-step recurrence ===========
        if 3 not in phases:
            return
        # v4: PE-accumulate chain (state in PSUM banks) + batched read
        # finalize. Per step t (state Mv_t in bank st[t%3]):
        #   PE   : G_t = lg9^T wd9 -> G-ring bank      (start/stop)
        #   ACT  : g_sb = copy(G_t)                    (PSUM -> SBUF)
        #   DVE  : S_t = reduce_m(prev) -> S ring      (off-chain)
        #   DVE  : new = prev * g_sb                   (chain)
        #   PE   : new += la9^T wd9                    (accum, chain)
        #   Pool : every RB steps, 3 batched TTs finalize reads:
        #          read_t = (S_t - S_{t+1} + a_t) * erecip_t
        # The accumulate relies on has_written=1 across each state bank,
        # arranged by a start=True zero-matmul before the loop.
        RB = 50
        state_fin = persist.tile([128, BL * M], F32)
        with tc.tile_pool(name="rec_ld", bufs=2) as rec_ld, \
             tc.tile_pool(name="rec_gw", bufs=4) as rec_gw, \
             tc.tile_pool(name="rec_stp", bufs=1) as rec_stp, \
             tc.tile_pool(name="rec_sm", bufs=2) as rec_sm, \
             tc.tile_pool(name="rec_st", bufs=1, space="PSUM") as rec_st, \
             tc.tile_pool(name="rec_ps", bufs=4, space="PSUM") as rec_ps:

            st = []
            for i in range(3):
                bank = rec_st.tile([128, BL * M], F32, name=f"stbank{i}")
                nc.tensor.matmul(bank, zeros1200[0:1, 0:128], zeros1200[0:1, 0:400],
                                 start=True, stop=True)
                st.append(bank)
            for b in range(BL):
                nc.scalar.copy(out=st[0][:, b * M:(b + 1) * M], in_=Mv0T_sb)

            # S ring, j-major: cols [j*BL, (j+1)*BL) hold S_{t0+j}
            S_ring = rec_stp.tile([128, BL * (RB + 1)], F32)
            ring_jb = S_ring.rearrange("p (j b) -> p j b", b=BL)
            a_tb = a_T.rearrange("p (b t) -> p t b", t=T_PAD)
            er_tb = erecip_T.rearrange("p (b t) -> p t b", t=T_PAD)
            rd_tb = reads_T.rearrange("p (b t) -> p t b", t=T_PAD)

            def finalize_reads(t0, nn):
                # reads for steps [t0, t0+nn) from S_ring slots 0..nn (Pool)
                d1 = rec_sm.tile([128, BL * RB], F32, tag="d1")
                d1v = d1.rearrange("p (j b) -> p j b", b=BL)
                nc.gpsimd.tensor_tensor(out=d1v[:, 0:nn, :], in0=ring_jb[:, 0:nn, :],
                                        in1=ring_jb[:, 1:nn + 1, :], op=OP.subtract)
                d2 = rec_sm.tile([128, BL * RB], F32, tag="d2")
                d2v = d2.rearrange("p (j b) -> p j b", b=BL)
                nc.gpsimd.tensor_tensor(out=d2v[:, 0:nn, :], in0=d1v[:, 0:nn, :],
                                        in1=a_tb[:, t0:t0 + nn, :], op=OP.add)
                nc.gpsimd.tensor_tensor(out=rd_tb[:, t0:t0 + nn, :],
                                        in0=d2v[:, 0:nn, :],
                                        in1=er_tb[:, t0:t0 + nn, :], op=OP.mult)
                # carry last S to slot 0 for the next block
                nc.gpsimd.tensor_copy(out=S_ring[:, 0:BL],
                                      in_=S_ring[:, nn * BL:(nn + 1) * BL])

            def reduce_S(t, bank):
                j = RB if (t % RB == 0 and t > 0) else t % RB
                nc.vector.tensor_reduce(
                    out=S_ring[:, j * BL:(j + 1) * BL],
                    in_=bank.rearrange("p (b m) -> p b m", b=BL),
                    axis=mybir.AxisListType.X, op=OP.add)

            SB = 4                       # sub-block: G matmuls run ahead
            for j0 in range(0, T, STEP_CHUNK):
                n = min(STEP_CHUNK, T - j0)
                wd_c = rec_ld.tile([9, STEP_CHUNK * 400], BF16, tag="wd_c")
                nc.sync.dma_start(out=wd_c[:, 0:n * 400],
                                  in_=_ap(wd9f[:], j0 * W9, [[400, 9], [W9, n], [1, 400]]))
                lg_c = rec_ld.tile([9, STEP_CHUNK * 128], BF16, tag="lg_c")
                nc.sync.dma_start(out=lg_c[:, 0:n * 128],
                                  in_=_ap(lg9f[:], j0 * L9, [[128, 9], [L9, n], [1, 128]]))
                la_c = rec_ld.tile([9, STEP_CHUNK * 128], BF16, tag="la_c")
                nc.sync.dma_start(out=la_c[:, 0:n * 128],
                                  in_=_ap(la9f[:], j0 * A9, [[128, 9], [A9, n], [1, 128]]))

                for s0 in range(0, n, SB):
                    gsbs = {}
                    for s in range(s0, min(s0 + SB, n)):
                        g_ps = rec_ps.tile([128, BL * M], F32, tag="g")
                        nc.tensor.matmul(g_ps, lg_c[:, s * 128:(s + 1) * 128],
                                         wd_c[:, s * 400:(s + 1) * 400],
                                         start=True, stop=True)
                        g_sb = rec_gw.tile([128, BL * M], F32, tag="g_sb")
                        nc.scalar.copy(out=g_sb, in_=g_ps)
                        gsbs[s] = g_sb
                    for s in range(s0, min(s0 + SB, n)):
                        t = j0 + s
                        prev, new = st[t % 3], st[(t + 1) % 3]
                        nc.vector.tensor_tensor(out=new, in0=prev, in1=gsbs[s],
                                                op=OP.mult)
                        nc.tensor.matmul(new, la_c[:, s * 128:(s + 1) * 128],
                                         wd_c[:, s * 400:(s + 1) * 400], start=False,
                                         stop=True, skip_group_check=True)
                        # S_t from the completed prev bank; AFTER the mult so
                        # the TR fills DVE slack while the PE accumulates WA
                        reduce_S(t, prev)
                        if t % RB == 0 and t > 0:
                            finalize_reads(t - RB, RB)
            # epilogue: S_500 -> slot 50, last fin block
            reduce_S(T, st[T % 3])
            finalize_reads(T - RB, RB)
            if dbg:
                nc.scalar.copy(out=state_fin, in_=st[T % 3])

        # =========== phase 4: output heads ===========
        if 4 not in phases:
            return
        # zero the padded t in [500, 512) columns of every batch row
        nc.vector.memset(
            reads_T.rearrange("p (b t) -> p b t", t=T_PAD)[:, :, T:T_PAD], 0.0)
        with tc.tile_pool(name="fin", bufs=2) as fin, \
             tc.tile_pool(name="finps", bufs=2, space="PSUM") as finps:
            for c in range(8):
                sl = slice(c * 512, (c + 1) * 512)
                f_ps = finps.tile([128, 512], F32, tag="f_ps")
                nc.tensor.matmul(f_ps, Wf_r, reads_T[:, sl], start=True, stop=False)
                nc.tensor.matmul(f_ps, Wf_k, k_T[:, sl], start=False, stop=True)
                nc.scalar.activation(out=f_T[:, sl], in_=f_ps, func=AF.Tanh, bias=bf_col)

            # heads: [2, 512] PSUM per chunk = {stu_logit; qd_logit}
            Wab0 = fin.tile([128, 2], F32, tag="wab0")
            nc.vector.memset(Wab0, 0.0)
            nc.sync.dma_start(out=Wab0[:, 0:1], in_=h["Wab"][:, :])
            W0d = fin.tile([128, 2], BF16, tag="w0d")
            nc.vector.memset(W0d, 0.0)
            nc.vector.tensor_copy(out=W0d[:, 1:2], in_=Wd_sb)
            comb = fin.tile([2, 1], F32, tag="comb")
            nc.vector.memset(comb, -1.0)
            nc.vector.memset(comb[0:1, :], 3.0)
            bias2 = fin.tile([2, 1], F32, tag="bias2")
            nc.sync.dma_start(out=bias2[0:1, :], in_=_ap(h["bab"][:], 0, [[1, 1], [1, 1]]))
            nc.sync.dma_start(out=bias2[1:2, :], in_=_ap(h["bd"][:], 0, [[1, 1], [1, 1]]))

            prob_row = fin.tile([1, NTOK], F32, tag="prob_row")
            for c in range(8):
                sl = slice(c * 512, (c + 1) * 512)
                hp = finps.tile([2, 512], F32, tag="hp")
                nc.tensor.matmul(hp, Wab0, f_T[:, sl], start=True, stop=False)
                nc.tensor.matmul(hp, W0d, k_T[:, sl], start=False, stop=True)
                ht = fin.tile([2, 512], F32, tag="ht")
                nc.scalar.activation(out=ht, in_=hp, func=AF.Tanh, bias=bias2)
                lg_ps = finps.tile([1, 512], F32, tag="lg_ps")
                nc.tensor.matmul(lg_ps, comb, ht, start=True, stop=True)
                nc.scalar.activation(out=prob_row[:, sl], in_=lg_ps, func=AF.Sigmoid)
            # prob_row[0, t*8+b] -> out[b, t]
            for b in range(BL):
                nc.sync.dma_start(out=out_h[b:b + 1, :],
                                  in_=prob_row[0:1, b * T_PAD:b * T_PAD + T])
            if dbg:
                for name, tile_ in (("dbg_kT", k_T), ("dbg_eT", e_T), ("dbg_aT", a_T),
                                    ("dbg_erecip", erecip_T), ("dbg_reads", reads_T),
                                    ("dbg_fT", f_T), ("dbg_state", state_fin),
                                    ("dbg_prob", prob_row)):
                    nc.sync.dma_start(out=dbg[name][:, :], in_=tile_)
                nc.sync.dma_start(out=dbg["dbg_wd9"][:, :, :], in_=wd9)
                nc.sync.dma_start(out=dbg["dbg_lg9"][:, :, :], in_=lg9)
                nc.sync.dma_start(out=dbg["dbg_la9"][:, :, :], in_=la9)


_NC = None
LAST_RESULT = None


def _get_nc():
    global _NC
    if _NC is None:
        _NC = build_program()
    return _NC


def kernel(**inputs):
    global LAST_RESULT
    from concourse.bass_utils import run_bass_kernel_spmd

    nc = _get_nc()
    names = ["concept_seq", "correct_seq", "embed_key", "embed_value", "Mk", "Mv0",
             "We", "be", "Wa", "ba", "Wf", "bf", "Wab", "bab", "Wd", "bd"]
    full = {k: np.ascontiguousarray(np.asarray(inputs[k])) for k in names}
    in_maps = []
    for i in range(NCORES):
        m = dict(full)
        m["concept_seq"] = np.ascontiguousarray(full["concept_seq"][i * BL:(i + 1) * BL])
        m["correct_seq"] = np.ascontiguousarray(full["correct_seq"][i * BL:(i + 1) * BL])
        in_maps.append(m)
    res = run_bass_kernel_spmd(nc, in_maps, core_ids=list(range(NCORES)))
    LAST_RESULT = res
    return np.concatenate([res.results[i]["out"] for i in range(NCORES)], axis=0)


if __name__ == "__main__":
    nc = build_program()
    print("build ok:", len(nc.m.functions[0].instructions) if hasattr(nc.m.functions[0], "instructions") else "n/a")



# revision 18
# speedup vs baseline: 1.0116x; 1.0116x over previous
"""DeepIRT (DKVMN) Trainium2 kernel.

Contract: kernel(**inputs) takes the FULL unsharded inputs of reference.py's
setup_inputs() and returns the full [64, 500] float32 output.

Strategy (8 NeuronCores, pure data parallel over batch):
  - each core handles BL=8 batch rows; tables/weights replicated.
  - precompute per core: gather k/v embeddings (indirect DMA), transpose to
    [d, token] layout with PE, compute w = softmax(k Mk^T), e = sigmoid(v We+be),
    a = tanh(v Wa+ba); pack per-step PE coefficient streams into DRAM:
      wd9[t]  = [9, 400]: row 0 = ones(400); row 1+b = w[b,t,:] placed in a
                block-diagonal at columns [b*50, b*50+50)
      lg9[t]  = [9, 128]: row 0 = ones(128); row 1+b = -e[b,t,:]
      la9[t]  = [9, 128]: row 0 = zeros;     row 1+b =  a[b,t,:]
  - recurrence over t (state Mv layout [128(d), 8b*50m] in SBUF):
      G  = lg9[t]^T @ wd9[t]  -> PSUM [128, 400] = 1 - w (x) e   (PE)
      WA = la9[t]^T @ wd9[t]  -> PSUM [128, 400] =      w (x) a  (PE)
      X  = Mv * G                                   (DVE tensor_tensor)
      Mv = X + WA                                   (DVE tensor_tensor)
      RX = reduce_m(X)          [128, 8]            (DVE tensor_reduce)
      read_t = (S_prev - RX) * (1/e_t)  (since reduce_m(Mv*(w e)) = e * read)
      S      = RX + a_t                 (since softmax rows sum to 1)
    1/e is computed exactly as 1 + exp(-z) from the sigmoid logits z.
  - final: f = tanh([reads, k] Wf + bf) via two accumulating matmuls on the
    [d, token] archives; stu/qd heads via [128,1] matmuls into an [8, 500]
    PSUM tile; predict = sigmoid(3*stu - qd) -> output [8, 500].

Tokens are ordered b-major: tok = b*T_PAD + t with T padded to T_PAD=512
(4096 tokens) so every 128-token tile is full and lies within one batch row;
padded slots use index 0 and are never read by the recurrence or the output
path.
"""

import numpy as np

import concourse.bass as bass
import concourse.bacc as bacc
import concourse.tile as tile
import concourse.mybir as mybir
from concourse.masks import make_identity

F32 = mybir.dt.float32
BF16 = mybir.dt.bfloat16
I32 = mybir.dt.int32
I16 = mybir.dt.int16
OP = mybir.AluOpType
AF = mybir.ActivationFunctionType

NUM_CONCEPT = 1000
D = 128
M = 50
B_FULL, T = 64, 500
NCORES = 8
BL = B_FULL // NCORES          # 8 batch rows per core
T_PAD = 512
NTOK = T_PAD * BL              # 4096 padded tokens, b-major: tok = b*T_PAD + t
NCH = NTOK // 128              # 32 gather/transpose chunks
W9 = 9 * 400                   # wd9 row stride in elements
L9 = 9 * 128
A9 = 9 * 128
STEP_CHUNK = 16                # recurrence steps loaded per DMA


def _ap(t, offset, dims):
    return bass.AP(t.tensor, offset, [list(d) for d in dims])


def _evac(nc, i, out, in_):
    """PSUM->SBUF copy, round-robined over DVE / ACT (Pool can't read PSUM)."""
    if i % 2 == 0:
        nc.vector.tensor_copy(out=out, in_=in_)
    else:
        nc.scalar.copy(out=out, in_=in_)


def build_program(debug_taps=False, phases=(1, 2, 3, 4)):
    nc = bacc.Bacc("TRN2", target_bir_lowering=False, debug=False)

    # ---------------- I/O ----------------
    h = {}
    h["concept_seq"] = nc.declare_dram_parameter("concept_seq", [BL, T], I32, isOutput=False)
    h["correct_seq"] = nc.declare_dram_parameter("correct_seq", [BL, T], I32, isOutput=False)
    h["embed_key"] = nc.declare_dram_parameter("embed_key", [NUM_CONCEPT, D], F32, isOutput=False)
    h["embed_value"] = nc.declare_dram_parameter("embed_value", [2 * NUM_CONCEPT, D], F32, isOutput=False)
    h["Mk"] = nc.declare_dram_parameter("Mk", [M, D], F32, isOutput=False)
    h["Mv0"] = nc.declare_dram_parameter("Mv0", [M, D], F32, isOutput=False)
    h["We"] = nc.declare_dram_parameter("We", [D, D], F32, isOutput=False)
    h["be"] = nc.declare_dram_parameter("be", [D], F32, isOutput=False)
    h["Wa"] = nc.declare_dram_parameter("Wa", [D, D], F32, isOutput=False)
    h["ba"] = nc.declare_dram_parameter("ba", [D], F32, isOutput=False)
    h["Wf"] = nc.declare_dram_parameter("Wf", [2 * D, D], F32, isOutput=False)
    h["bf"] = nc.declare_dram_parameter("bf", [D], F32, isOutput=False)
    h["Wab"] = nc.declare_dram_parameter("Wab", [D, 1], F32, isOutput=False)
    h["bab"] = nc.declare_dram_parameter("bab", [1], F32, isOutput=False)
    h["Wd"] = nc.declare_dram_parameter("Wd", [D, 1], F32, isOutput=False)
    h["bd"] = nc.declare_dram_parameter("bd", [1], F32, isOutput=False)
    out_h = nc.declare_dram_parameter("out", [BL, T], F32, isOutput=True)
    dbg = {}
    if debug_taps:
        for n in ("dbg_eT", "dbg_aT", "dbg_erecip", "dbg_fT"):
            dbg[n] = nc.declare_dram_parameter(n, [128, NTOK], F32, isOutput=True)
        for n in ("dbg_kT", "dbg_reads"):
            dbg[n] = nc.declare_dram_parameter(n, [128, NTOK], BF16, isOutput=True)
        dbg["dbg_state"] = nc.declare_dram_parameter("dbg_state", [128, BL * M], F32, isOutput=True)
        dbg["dbg_prob"] = nc.declare_dram_parameter("dbg_prob", [1, NTOK], F32, isOutput=True)
        dbg["dbg_wd9"] = nc.declare_dram_parameter("dbg_wd9", [T_PAD, 9, 400], BF16, isOutput=True)
        dbg["dbg_lg9"] = nc.declare_dram_parameter("dbg_lg9", [T_PAD, 9, 128], BF16, isOutput=True)
        dbg["dbg_la9"] = nc.declare_dram_parameter("dbg_la9", [T_PAD, 9, 128], BF16, isOutput=True)

    with tile.TileContext(nc) as tc:
        _emit(nc, tc, h, out_h, dbg, phases)
    nc.finalize()
    return nc


def _emit(nc, tc, h, out_h, dbg=None, phases=(1, 2, 3, 4)):
    from contextlib import ExitStack

    ctx = ExitStack()
    with ctx:
        # ---- pools ----
        persist = ctx.enter_context(tc.tile_pool(name="persist", bufs=1))
        dram = ctx.enter_context(tc.tile_pool(name="dram", bufs=1, space="DRAM"))

        # DRAM coefficient streams
        wd9 = dram.tile([T_PAD, 9, 400], BF16)
        lg9 = dram.tile([T_PAD, 9, 128], BF16)
        la9 = dram.tile([T_PAD, 9, 128], BF16)

        # persistent SBUF archives ([d, token] layouts, t-major tokens)
        k_T = persist.tile([128, NTOK], BF16)
        v_T = persist.tile([128, NTOK], BF16)
        e_T = persist.tile([128, NTOK], BF16)
        erecip_T = persist.tile([128, NTOK], F32)
        a_T = persist.tile([128, NTOK], BF16)
        reads_T = persist.tile([128, NTOK], BF16)
        f_T = persist.tile([128, NTOK], F32)

        # small persistent constants / weights
        ident = persist.tile([128, 128], F32)
        make_identity(nc, ident)
        ident_bf = persist.tile([128, 128], BF16)
        nc.vector.tensor_copy(out=ident_bf, in_=ident)
        ones128 = persist.tile([128, 128], BF16)
        nc.vector.memset(ones128, 1.0)
        ones400 = persist.tile([128, 400], BF16)
        nc.vector.memset(ones400, 1.0)
        zeros1200 = persist.tile([128, 1200], BF16)
        nc.vector.memset(zeros1200, 0.0)
        zeros400f = persist.tile([1, 400], F32)
        nc.vector.memset(zeros400f, 0.0)

        We_f32 = persist.tile([128, 128], F32)
        nc.sync.dma_start(out=We_f32, in_=h["We"][:, :])
        We_sb = persist.tile([128, 128], BF16)
        nc.vector.tensor_copy(out=We_sb, in_=We_f32)
        Wa_f32 = persist.tile([128, 128], F32)
        nc.sync.dma_start(out=Wa_f32, in_=h["Wa"][:, :])
        Wa_sb = persist.tile([128, 128], BF16)
        nc.vector.tensor_copy(out=Wa_sb, in_=Wa_f32)
        Wf_r32 = persist.tile([128, 128], F32)
        nc.sync.dma_start(out=Wf_r32, in_=h["Wf"][0:128, :])
        Wf_r = persist.tile([128, 128], BF16)
        nc.vector.tensor_copy(out=Wf_r, in_=Wf_r32)
        Wf_k32 = persist.tile([128, 128], F32)
        nc.sync.dma_start(out=Wf_k32, in_=h["Wf"][128:256, :])
        Wf_k = persist.tile([128, 128], BF16)
        nc.vector.tensor_copy(out=Wf_k, in_=Wf_k32)
        Wab_sb = persist.tile([128, 1], F32)
        nc.sync.dma_start(out=Wab_sb, in_=h["Wab"][:, :])
        Wd_sb = persist.tile([128, 1], F32)
        nc.sync.dma_start(out=Wd_sb, in_=h["Wd"][:, :])
        Mk_sb = persist.tile([50, 128], F32)
        nc.sync.dma_start(out=Mk_sb, in_=h["Mk"][:, :])
        Mv0_sb = persist.tile([50, 128], F32)
        nc.sync.dma_start(out=Mv0_sb, in_=h["Mv0"][:, :])

        def col(name, n=128):
            t = persist.tile([n, 1], F32, name=name)
            nc.sync.dma_start(out=t, in_=_ap(h[name[:-4]][:], 0, [[1, n], [1, 1]]))
            return t

        be_col = col("be_col")
        ba_col = col("ba_col")
        bf_col = col("bf_col")
        neg_be_col = persist.tile([128, 1], F32)
        nc.vector.tensor_scalar(out=neg_be_col, in0=be_col, scalar1=-1.0, scalar2=None, op0=OP.mult)

        # ---- stream-layout views ----
        wd9f = wd9.rearrange("t r c -> t (r c)")
        lg9f = lg9.rearrange("t r c -> t (r c)")
        la9f = la9.rearrange("t r c -> t (r c)")

        # ---- transpose Mv0 and Mk once; Mv0^T kept in SBUF for state init ----
        Mv0T_sb = persist.tile([128, 50], F32)
        MkT_sb = persist.tile([128, 50], BF16)
        with tc.tile_pool(name="init_ps", bufs=1, space="PSUM") as initp:
            mv0t = initp.tile([128, 50], F32)
            nc.tensor.transpose(mv0t, Mv0_sb, ident[0:50, 0:50])
            nc.any.tensor_copy(out=Mv0T_sb, in_=mv0t)
            mkt_ps = initp.tile([128, 50], F32)
            nc.tensor.transpose(mkt_ps, Mk_sb, ident[0:50, 0:50])
            nc.any.tensor_copy(out=MkT_sb, in_=mkt_ps)

        # =========== phases 1+2 interleaved ===========
        if 1 not in phases or 2 not in phases:
            return
        idxk_dram = dram.tile([NTOK], I32)
        idxv_dram = dram.tile([NTOK], I32)
        ek_bf = dram.tile([NUM_CONCEPT, 128], BF16)
        ev_bf = dram.tile([2 * NUM_CONCEPT, 128], BF16)
        with tc.tile_pool(name="ph1", bufs=1) as ph1, \
             tc.tile_pool(name="ph1t", bufs=4) as ph1t, \
             tc.tile_pool(name="ph1ps", bufs=2, space="PSUM") as ph1ps, \
             tc.tile_pool(name="ph2", bufs=3) as ph2, \
             tc.tile_pool(name="ph2ps", bufs=1, space="PSUM") as ph2ps:

            # ---- index prep FIRST so the gather queue starts early ----
            cseq = ph1.tile([8, T_PAD], I32)
            crse = ph1.tile([8, T_PAD], I32)
            nc.vector.memset(cseq, 0)
            nc.vector.memset(crse, 0)
            nc.sync.dma_start(out=cseq[:, 0:T], in_=h["concept_seq"][:, :])
            nc.scalar.dma_start(out=crse[:, 0:T], in_=h["correct_seq"][:, :])

            # x = concept + 1000*correct (exact in fp32, cast back to ints)
            cseq_f = ph1.tile([8, T_PAD], F32)
            nc.vector.tensor_copy(out=cseq_f, in_=cseq)
            crse_f = ph1.tile([8, T_PAD], F32)
            nc.vector.tensor_copy(out=crse_f, in_=crse)
            x_f = ph1.tile([8, T_PAD], F32)
            nc.vector.scalar_tensor_tensor(out=x_f, in0=crse_f, scalar=float(NUM_CONCEPT),
                                           in1=cseq_f, op0=OP.mult, op1=OP.add)
            x_i = ph1.tile([8, T_PAD], I32)
            nc.vector.tensor_copy(out=x_i, in_=x_f)

            # bounce through DRAM to rewrap indices token-major: chunk g's
            # 128 tokens land as column g of a [128, NCH] tile
            ckw = ph1.tile([128, NCH], I32)
            xvw = ph1.tile([128, NCH], I32)
            for srct, drt, dstt, eng in ((x_i, idxv_dram, xvw, nc.scalar),
                                         (cseq, idxk_dram, ckw, nc.sync)):
                eng.dma_start(out=_ap(drt[:], 0, [[T_PAD, 8], [1, T_PAD]]), in_=srct)
                eng.dma_start(out=dstt, in_=_ap(drt[:], 0, [[1, 128], [128, NCH]]))

            # bf16 copies of the embedding tables in DRAM (halves gather
            # bytes); value table first so v-gathers start earliest.
            # 512 rows per DMA: row r=(c*128+p) -> SBUF [p, c*128+d]
            for src_h, dst_t, nrows in ((h["embed_value"], ev_bf, 2 * NUM_CONCEPT),
                                        (h["embed_key"], ek_bf, NUM_CONCEPT)):
                for r0 in range(0, nrows, 512):
                    nch4 = min(4, (nrows - r0 + 127) // 128)
                    nlast = min(128, nrows - r0 - (nch4 - 1) * 128)
                    wide = (nch4 - 1) * 128 + nlast
                    tl = ph1t.tile([128, 512], F32, tag="tcv")
                    eng = (nc.sync, nc.scalar)[(r0 // 512) % 2]
                    if nlast == 128:
                        eng.dma_start(
                            out=tl[:, 0:nch4 * 128].rearrange("p (c d) -> p c d", c=nch4),
                            in_=_ap(src_h[:, :], r0 * 128,
                                    [[128, 128], [128 * 128, nch4], [1, 128]]))
                    else:
                        if nch4 > 1:
                            eng.dma_start(
                                out=tl[:, 0:(nch4 - 1) * 128].rearrange("p (c d) -> p c d", c=nch4 - 1),
                                in_=_ap(src_h[:, :], r0 * 128,
                                        [[128, 128], [128 * 128, nch4 - 1], [1, 128]]))
                        eng.dma_start(
                            out=tl[0:nlast, (nch4 - 1) * 128:nch4 * 128],
                            in_=_ap(src_h[:, :], (r0 + (nch4 - 1) * 128) * 128,
                                    [[128, nlast], [1, 128]]))
                    tb = ph1t.tile([128, 512], BF16, tag="tcb")
                    nc.vector.tensor_copy(out=tb, in_=tl)
                    if nlast == 128:
                        eng.dma_start(
                            out=_ap(dst_t[:, :], r0 * 128,
                                    [[128, 128], [128 * 128, nch4], [1, 128]]),
                            in_=tb[:, 0:nch4 * 128].rearrange("p (c d) -> p c d", c=nch4))
                    else:
                        if nch4 > 1:
                            eng.dma_start(
                                out=_ap(dst_t[:, :], r0 * 128,
                                        [[128, 128], [128 * 128, nch4 - 1], [1, 128]]),
                                in_=tb[:, 0:(nch4 - 1) * 128].rearrange("p (c d) -> p c d", c=nch4 - 1))
                        eng.dma_start(
                            out=_ap(dst_t[:, :], (r0 + (nch4 - 1) * 128) * 128,
                                    [[128, nlast], [1, 128]]),
                            in_=tb[0:nlast, (nch4 - 1) * 128:nch4 * 128])

            # ---- fill DRAM streams (sync/scalar only; gpsimd queue is for
            # gathers) ----
            fillqs = (nc.sync, nc.sync)
            fq = 0
            for r0 in range(0, T_PAD, 128):
                for c0 in range(0, 3600, 1200):
                    fillqs[fq % 2].dma_start(out=wd9f[r0:r0 + 128, c0:c0 + 1200], in_=zeros1200)
                    fq += 1
                fillqs[fq % 2].dma_start(out=wd9[r0:r0 + 128, 0, :], in_=ones400); fq += 1
                fillqs[fq % 2].dma_start(out=lg9[r0:r0 + 128, 0, :], in_=ones128); fq += 1
                # lg rows 1-8 written by e pass; zero them anyway for padded tail rows
                fillqs[fq % 2].dma_start(out=lg9f[r0:r0 + 128, 128:1152],
                                         in_=zeros1200[:, 0:1024]); fq += 1
                fillqs[fq % 2].dma_start(out=la9f[r0:r0 + 128, 0:1152],
                                         in_=zeros1200[:, 0:1152]); fq += 1

            def gather_chunk(g, table, idx_tile, dst, tag):
                rows = ph1t.tile([128, 128], BF16, tag=tag)
                nc.gpsimd.indirect_dma_start(
                    out=rows, out_offset=None, in_=table,
                    in_offset=bass.IndirectOffsetOnAxis(ap=idx_tile[:, g:g + 1], axis=0))
                tps = ph1ps.tile([128, 128], BF16, tag="gt")
                nc.tensor.transpose(tps, rows, ident_bf)
                _evac(nc, g, dst[:, 128 * g:128 * (g + 1)], tps)

            # ---- loop A: v-gathers interleaved with e/a slice compute ----
            for c in range(8):
                for g in range(4 * c, 4 * c + 4):
                    gather_chunk(g, ev_bf[:, :], xvw, v_T, "rv")
                sl = slice(c * 512, (c + 1) * 512)
                elog = ph2ps.tile([128, 512], F32, tag="ealog", bufs=2)
                nc.tensor.matmul(elog, We_sb, v_T[:, sl], start=True, stop=True)
                nc.scalar.activation(out=e_T[:, sl], in_=elog, func=AF.Sigmoid, bias=be_col)
                etmp = ph2.tile([128, 512], F32, tag="etmp")
                nc.scalar.activation(out=etmp, in_=elog, func=AF.Exp,
                                     bias=neg_be_col, scale=-1.0)
                nc.vector.tensor_scalar(out=erecip_T[:, sl], in0=etmp, scalar1=1.0,
                                        scalar2=None, op0=OP.add)
                alog = ph2ps.tile([128, 512], F32, tag="ealog", bufs=2)
                nc.tensor.matmul(alog, Wa_sb, v_T[:, sl], start=True, stop=True)
                nc.scalar.activation(out=a_T[:, sl], in_=alog, func=AF.Tanh, bias=ba_col)

            # ---- loop B: k-gathers interleaved with w softmax + lg/la rows,
            # t0-major, b inner; per-chunk results land in staging tiles that
            # flush as 3 wide DMAs per t0 block (12 scatter DMAs total) ----
            stg = {}
            for i in range(NCH):
                t0i, b = i // 8, i % 8
                p = 4 * b + t0i
                if b == 0:
                    stg_w = ph2.tile([128, 400], BF16, tag="stgw", bufs=2, name="stg_w")
                    stg_e = ph2.tile([128, 1024], BF16, tag="stge", bufs=2, name="stg_e")
                    stg_a = ph2.tile([128, 1024], BF16, tag="stga", bufs=2, name="stg_a")
                    stg = {"w": stg_w, "e": stg_e, "a": stg_a}
                gather_chunk(p, ek_bf[:, :], ckw, k_T, "rk")
                sl = slice(p * 128, (p + 1) * 128)
                wlog = ph2ps.tile([128, 50], F32, tag="wlog", bufs=1)
                nc.tensor.matmul(wlog, k_T[:, sl], MkT_sb, start=True, stop=True)
                negmax = ph2.tile([128, 1], F32, tag="negmax")
                nc.vector.tensor_reduce(out=negmax, in_=wlog, axis=mybir.AxisListType.X,
                                        op=OP.max, negate=True)
                wexp = ph2.tile([128, 50], F32, tag="wexp")
                sumexp = ph2.tile([128, 1], F32, tag="sumexp")
                nc.scalar.activation(out=wexp, in_=wlog, func=AF.Exp, bias=negmax,
                                     accum_out=sumexp)
                rsum = ph2.tile([128, 1], F32, tag="rsum")
                nc.vector.reciprocal(out=rsum, in_=sumexp)
                nc.vector.tensor_scalar(out=stg["w"][:, b * 50:(b + 1) * 50], in0=wexp,
                                        scalar1=rsum, scalar2=None, op0=OP.mult)

                # e rows -> lg9 rows 1..8 (negated); a rows -> la9 rows 1..8
                ert = ph2ps.tile([128, 128], BF16, tag="eat", bufs=2)
                nc.tensor.transpose(ert, e_T[:, sl], ident_bf)
                if p % 2 == 0:
                    nc.vector.tensor_scalar(out=stg["e"][:, b * 128:(b + 1) * 128],
                                            in0=ert, scalar1=-1.0, scalar2=None,
                                            op0=OP.mult)
                else:
                    nc.scalar.mul(out=stg["e"][:, b * 128:(b + 1) * 128], in_=ert,
                                  mul=-1.0)
                art = ph2ps.tile([128, 128], BF16, tag="eat", bufs=2)
                nc.tensor.transpose(art, a_T[:, sl], ident_bf)
                if p % 2 == 0:
                    nc.scalar.copy(out=stg["a"][:, b * 128:(b + 1) * 128], in_=art)
                else:
                    nc.vector.tensor_copy(out=stg["a"][:, b * 128:(b + 1) * 128],
                                          in_=art)

                if b == 7:
                    t0 = 128 * t0i
                    nc.sync.dma_start(
                        out=_ap(wd9f[:], t0 * W9 + 400, [[W9, 128], [450, 8], [1, 50]]),
                        in_=stg["w"].rearrange("p (b c) -> p b c", b=8))
                    nc.scalar.dma_start(out=lg9f[t0:t0 + 128, 128:1152], in_=stg["e"])
                    nc.sync.dma_start(out=la9f[t0:t0 + 128, 128:1152], in_=stg["a"])

        # =========== phase 3: the 500-step recurrence ===========
        if 3 not in phases:
            return
        # v4: PE-accumulate chain (state in PSUM banks) + batched read
        # finalize. Per step t (state Mv_t in bank st[t%3]):
        #   PE   : G_t = lg9^T wd9 -> G-ring bank      (start/stop)
        #   ACT  : g_sb = copy(G_t)                    (PSUM -> SBUF)
        #   DVE  : S_t = reduce_m(prev) -> S ring      (off-chain)
        #   DVE  : new = prev * g_sb                   (chain)
        #   PE   : new += la9^T wd9                    (accum, chain)
        #   Pool : every RB steps, 3 batched TTs finalize reads:
        #          read_t = (S_t - S_{t+1} + a_t) * erecip_t
        # The accumulate relies on has_written=1 across each state bank,
        # arranged by a start=True zero-matmul before the loop.
        RB = 50
        state_fin = persist.tile([128, BL * M], F32)
        with tc.tile_pool(name="rec_ld", bufs=2) as rec_ld, \
             tc.tile_pool(name="rec_gw", bufs=4) as rec_gw, \
             tc.tile_pool(name="rec_stp", bufs=1) as rec_stp, \
             tc.tile_pool(name="rec_sm", bufs=2) as rec_sm, \
             tc.tile_pool(name="rec_st", bufs=1, space="PSUM") as rec_st, \
             tc.tile_pool(name="rec_ps", bufs=4, space="PSUM") as rec_ps:

            st = []
            for i in range(3):
                bank = rec_st.tile([128, BL * M], F32, name=f"stbank{i}")
                nc.tensor.matmul(bank, zeros1200[0:1, 0:128], zeros1200[0:1, 0:400],
                                 start=True, stop=True)
                st.append(bank)
            for b in range(BL):
                nc.scalar.copy(out=st[0][:, b * M:(b + 1) * M], in_=Mv0T_sb)

            # S ring, j-major: cols [j*BL, (j+1)*BL) hold S_{t0+j}
            S_ring = rec_stp.tile([128, BL * (RB + 1)], F32)
            ring_jb = S_ring.rearrange("p (j b) -> p j b", b=BL)
            a_tb = a_T.rearrange("p (b t) -> p t b", t=T_PAD)
            er_tb = erecip_T.rearrange("p (b t) -> p t b", t=T_PAD)
            rd_tb = reads_T.rearrange("p (b t) -> p t b", t=T_PAD)

            def finalize_reads(t0, nn):
                # reads for steps [t0, t0+nn) from S_ring slots 0..nn (Pool)
                d1 = rec_sm.tile([128, BL * RB], F32, tag="d1")
                d1v = d1.rearrange("p (j b) -> p j b", b=BL)
                nc.gpsimd.tensor_tensor(out=d1v[:, 0:nn, :], in0=ring_jb[:, 0:nn, :],
                                        in1=ring_jb[:, 1:nn + 1, :], op=OP.subtract)
                d2 = rec_sm.tile([128, BL * RB], F32, tag="d2")
                d2v = d2.rearrange("p (j b) -> p j b", b=BL)
                nc.gpsimd.tensor_tensor(out=d2v[:, 0:nn, :], in0=d1v[:, 0:nn, :],
                                        in1=a_tb[:, t0:t0 + nn, :], op=OP.add)
                nc.gpsimd.tensor_tensor(out=rd_tb[:, t0:t0 + nn, :],
                                        in0=d2v[:, 0:nn, :],
                                        in1=er_tb[:, t0:t0 + nn, :], op=OP.mult)
                # carry last S to slot 0 for the next block
                nc.gpsimd.tensor_copy(out=S_ring[:, 0:BL],
                                      in_=S_ring[:, nn * BL:(nn + 1) * BL])

            def reduce_S(t, bank):
                j = RB if (t % RB == 0 and t > 0) else t % RB
                nc.vector.tensor_reduce(
                    out=S_ring[:, j * BL:(j + 1) * BL],
                    in_=bank.rearrange("p (b m) -> p b m", b=BL),
                    axis=mybir.AxisListType.X, op=OP.add)

            SB = 4                       # sub-block: G matmuls run ahead
            for j0 in range(0, T, STEP_CHUNK):
                n = min(STEP_CHUNK, T - j0)
                wd_c = rec_ld.tile([9, STEP_CHUNK * 400], BF16, tag="wd_c")
                nc.sync.dma_start(out=wd_c[:, 0:n * 400],
                                  in_=_ap(wd9f[:], j0 * W9, [[400, 9], [W9, n], [1, 400]]))
                lg_c = rec_ld.tile([9, STEP_CHUNK * 128], BF16, tag="lg_c")
                nc.sync.dma_start(out=lg_c[:, 0:n * 128],
                                  in_=_ap(lg9f[:], j0 * L9, [[128, 9], [L9, n], [1, 128]]))
                la_c = rec_ld.tile([9, STEP_CHUNK * 128], BF16, tag="la_c")
                nc.sync.dma_start(out=la_c[:, 0:n * 128],
                                  in_=_ap(la9f[:], j0 * A9, [[128, 9], [A9, n], [1, 128]]))

                for s0 in range(0, n, SB):
                    gsbs = {}
                    for s in range(s0, min(s0 + SB, n)):
                        g_ps = rec_ps.tile([128, BL * M], F32, tag="g")
                        nc.tensor.matmul(g_ps, lg_c[:, s * 128:(s + 1) * 128],
                                         wd_c[:, s * 400:(s + 1) * 400],
                                         start=True, stop=True)
                        g_sb = rec_gw.tile([128, BL * M], F32, tag="g_sb")
                        nc.scalar.copy(out=g_sb, in_=g_ps)
                        gsbs[s] = g_sb
                    for s in range(s0, min(s0 + SB, n)):
                        t = j0 + s
                        prev, new = st[t % 3], st[(t + 1) % 3]
                        nc.vector.tensor_tensor(out=new, in0=prev, in1=gsbs[s],
                                                op=OP.mult)
                        nc.tensor.matmul(new, la_c[:, s * 128:(s + 1) * 128],
                                         wd_c[:, s * 400:(s + 1) * 400], start=False,
                                         stop=True, skip_group_check=True)
                        # S_t from the completed prev bank; AFTER the mult so
                        # the TR fills DVE slack while the PE accumulates WA
                        reduce_S(t, prev)
                        if t % RB == 0 and t > 0:
                            finalize_reads(t - RB, RB)
            # epilogue: S_500 -> slot 50, last fin block
            reduce_S(T, st[T % 3])
            finalize_reads(T - RB, RB)
            if dbg:
                nc.scalar.copy(out=state_fin, in_=st[T % 3])

        # =========== phase 4: output heads ===========
        if 4 not in phases:
            return
        # zero the padded t in [500, 512) columns of every batch row
        nc.vector.memset(
            reads_T.rearrange("p (b t) -> p b t", t=T_PAD)[:, :, T:T_PAD], 0.0)
        with tc.tile_pool(name="fin", bufs=2) as fin, \
             tc.tile_pool(name="finps", bufs=2, space="PSUM") as finps:
            for c in range(8):
                sl = slice(c * 512, (c + 1) * 512)
                f_ps = finps.tile([128, 512], F32, tag="f_ps")
                nc.tensor.matmul(f_ps, Wf_r, reads_T[:, sl], start=True, stop=False)
                nc.tensor.matmul(f_ps, Wf_k, k_T[:, sl], start=False, stop=True)
                nc.scalar.activation(out=f_T[:, sl], in_=f_ps, func=AF.Tanh, bias=bf_col)

            # heads: [2, 512] PSUM per chunk = {stu_logit; qd_logit}
            Wab0 = fin.tile([128, 2], F32, tag="wab0")
            nc.vector.memset(Wab0, 0.0)
            nc.sync.dma_start(out=Wab0[:, 0:1], in_=h["Wab"][:, :])
            W0d = fin.tile([128, 2], BF16, tag="w0d")
            nc.vector.memset(W0d, 0.0)
            nc.vector.tensor_copy(out=W0d[:, 1:2], in_=Wd_sb)
            comb = fin.tile([2, 1], F32, tag="comb")
            nc.vector.memset(comb, -1.0)
            nc.vector.memset(comb[0:1, :], 3.0)
            bias2 = fin.tile([2, 1], F32, tag="bias2")
            nc.sync.dma_start(out=bias2[0:1, :], in_=_ap(h["bab"][:], 0, [[1, 1], [1, 1]]))
            nc.sync.dma_start(out=bias2[1:2, :], in_=_ap(h["bd"][:], 0, [[1, 1], [1, 1]]))

            prob_row = fin.tile([1, NTOK], F32, tag="prob_row")
            for c in range(8):
                sl = slice(c * 512, (c + 1) * 512)
                hp = finps.tile([2, 512], F32, tag="hp")
                nc.tensor.matmul(hp, Wab0, f_T[:, sl], start=True, stop=False)
                nc.tensor.matmul(hp, W0d, k_T[:, sl], start=False, stop=True)
                ht = fin.tile([2, 512], F32, tag="ht")
                nc.scalar.activation(out=ht, in_=hp, func=AF.Tanh, bias=bias2)
                lg_ps = finps.tile([1, 512], F32, tag="lg_ps")
                nc.tensor.matmul(lg_ps, comb, ht, start=True, stop=True)
                nc.scalar.activation(out=prob_row[:, sl], in_=lg_ps, func=AF.Sigmoid)
            # prob_row[0, t*8+b] -> out[b, t]
            for b in range(BL):
                nc.sync.dma_start(out=out_h[b:b + 1, :],
                                  in_=prob_row[0:1, b * T_PAD:b * T_PAD + T])
            if dbg:
                for name, tile_ in (("dbg_kT", k_T), ("dbg_eT", e_T), ("dbg_aT", a_T),
                                    ("dbg_erecip", erecip_T), ("dbg_reads", reads_T),
                                    ("dbg_fT", f_T), ("dbg_state", state_fin),
                                    ("dbg_prob", prob_row)):
                    nc.sync.dma_start(out=dbg[name][:, :], in_=tile_)
                nc.sync.dma_start(out=dbg["dbg_wd9"][:, :, :], in_=wd9)
                nc.sync.dma_start(out=dbg["dbg_lg9"][:, :, :], in_=lg9)
                nc.sync.dma_start(out=dbg["dbg_la9"][:, :, :], in_=la9)


_NC = None
LAST_RESULT = None


def _get_nc():
    global _NC
    if _NC is None:
        _NC = build_program()
    return _NC


def kernel(**inputs):
    global LAST_RESULT
    from concourse.bass_utils import run_bass_kernel_spmd

    nc = _get_nc()
    names = ["concept_seq", "correct_seq", "embed_key", "embed_value", "Mk", "Mv0",
             "We", "be", "Wa", "ba", "Wf", "bf", "Wab", "bab", "Wd", "bd"]
    full = {k: np.ascontiguousarray(np.asarray(inputs[k])) for k in names}
    in_maps = []
    for i in range(NCORES):
        m = dict(full)
        m["concept_seq"] = np.ascontiguousarray(full["concept_seq"][i * BL:(i + 1) * BL])
        m["correct_seq"] = np.ascontiguousarray(full["correct_seq"][i * BL:(i + 1) * BL])
        in_maps.append(m)
    res = run_bass_kernel_spmd(nc, in_maps, core_ids=list(range(NCORES)))
    LAST_RESULT = res
    return np.concatenate([res.results[i]["out"] for i in range(NCORES)], axis=0)


if __name__ == "__main__":
    nc = build_program()
    print("build ok:", len(nc.m.functions[0].instructions) if hasattr(nc.m.functions[0], "instructions") else "n/a")



# revision 21
# speedup vs baseline: 1.0295x; 1.0177x over previous
"""DeepIRT (DKVMN) Trainium2 kernel.

Contract: kernel(**inputs) takes the FULL unsharded inputs of reference.py's
setup_inputs() and returns the full [64, 500] float32 output.

Strategy (8 NeuronCores, pure data parallel over batch):
  - each core handles BL=8 batch rows; tables/weights replicated.
  - precompute per core: gather k/v embeddings (indirect DMA), transpose to
    [d, token] layout with PE, compute w = softmax(k Mk^T), e = sigmoid(v We+be),
    a = tanh(v Wa+ba); pack per-step PE coefficient streams into DRAM:
      wd9[t]  = [9, 400]: row 0 = ones(400); row 1+b = w[b,t,:] placed in a
                block-diagonal at columns [b*50, b*50+50)
      lg9[t]  = [9, 128]: row 0 = ones(128); row 1+b = -e[b,t,:]
      la9[t]  = [9, 128]: row 0 = zeros;     row 1+b =  a[b,t,:]
  - recurrence over t (state Mv layout [128(d), 8b*50m] in SBUF):
      G  = lg9[t]^T @ wd9[t]  -> PSUM [128, 400] = 1 - w (x) e   (PE)
      WA = la9[t]^T @ wd9[t]  -> PSUM [128, 400] =      w (x) a  (PE)
      X  = Mv * G                                   (DVE tensor_tensor)
      Mv = X + WA                                   (DVE tensor_tensor)
      RX = reduce_m(X)          [128, 8]            (DVE tensor_reduce)
      read_t = (S_prev - RX) * (1/e_t)  (since reduce_m(Mv*(w e)) = e * read)
      S      = RX + a_t                 (since softmax rows sum to 1)
    1/e is computed exactly as 1 + exp(-z) from the sigmoid logits z.
  - final: f = tanh([reads, k] Wf + bf) via two accumulating matmuls on the
    [d, token] archives; stu/qd heads via [128,1] matmuls into an [8, 500]
    PSUM tile; predict = sigmoid(3*stu - qd) -> output [8, 500].

Tokens are ordered b-major: tok = b*T_PAD + t with T padded to T_PAD=512
(4096 tokens) so every 128-token tile is full and lies within one batch row;
padded slots use index 0 and are never read by the recurrence or the output
path.
"""

import numpy as np

import concourse.bass as bass
import concourse.bacc as bacc
import concourse.tile as tile
import concourse.mybir as mybir
from concourse.masks import make_identity

F32 = mybir.dt.float32
BF16 = mybir.dt.bfloat16
I32 = mybir.dt.int32
I16 = mybir.dt.int16
OP = mybir.AluOpType
AF = mybir.ActivationFunctionType

NUM_CONCEPT = 1000
D = 128
M = 50
B_FULL, T = 64, 500
NCORES = 8
BL = B_FULL // NCORES          # 8 batch rows per core
T_PAD = 512
NTOK = T_PAD * BL              # 4096 padded tokens, b-major: tok = b*T_PAD + t
NCH = NTOK // 128              # 32 gather/transpose chunks
W9 = 9 * 400                   # wd9 row stride in elements
L9 = 9 * 128
A9 = 9 * 128
STEP_CHUNK = 16                # recurrence steps loaded per DMA


def _ap(t, offset, dims):
    return bass.AP(t.tensor, offset, [list(d) for d in dims])


def _evac(nc, i, out, in_):
    """PSUM->SBUF copy, round-robined over DVE / ACT (Pool can't read PSUM)."""
    if i % 2 == 0:
        nc.vector.tensor_copy(out=out, in_=in_)
    else:
        nc.scalar.copy(out=out, in_=in_)


def build_program(debug_taps=False, phases=(1, 2, 3, 4)):
    nc = bacc.Bacc("TRN2", target_bir_lowering=False, debug=False)

    # ---------------- I/O ----------------
    h = {}
    h["concept_seq"] = nc.declare_dram_parameter("concept_seq", [BL, T], I32, isOutput=False)
    h["correct_seq"] = nc.declare_dram_parameter("correct_seq", [BL, T], I32, isOutput=False)
    h["embed_key"] = nc.declare_dram_parameter("embed_key", [NUM_CONCEPT, D], F32, isOutput=False)
    h["embed_value"] = nc.declare_dram_parameter("embed_value", [2 * NUM_CONCEPT, D], F32, isOutput=False)
    h["Mk"] = nc.declare_dram_parameter("Mk", [M, D], F32, isOutput=False)
    h["Mv0"] = nc.declare_dram_parameter("Mv0", [M, D], F32, isOutput=False)
    h["We"] = nc.declare_dram_parameter("We", [D, D], F32, isOutput=False)
    h["be"] = nc.declare_dram_parameter("be", [D], F32, isOutput=False)
    h["Wa"] = nc.declare_dram_parameter("Wa", [D, D], F32, isOutput=False)
    h["ba"] = nc.declare_dram_parameter("ba", [D], F32, isOutput=False)
    h["Wf"] = nc.declare_dram_parameter("Wf", [2 * D, D], F32, isOutput=False)
    h["bf"] = nc.declare_dram_parameter("bf", [D], F32, isOutput=False)
    h["Wab"] = nc.declare_dram_parameter("Wab", [D, 1], F32, isOutput=False)
    h["bab"] = nc.declare_dram_parameter("bab", [1], F32, isOutput=False)
    h["Wd"] = nc.declare_dram_parameter("Wd", [D, 1], F32, isOutput=False)
    h["bd"] = nc.declare_dram_parameter("bd", [1], F32, isOutput=False)
    out_h = nc.declare_dram_parameter("out", [BL, T], F32, isOutput=True)
    dbg = {}
    if debug_taps:
        for n in ("dbg_eT", "dbg_aT", "dbg_erecip", "dbg_fT"):
            dbg[n] = nc.declare_dram_parameter(n, [128, NTOK], F32, isOutput=True)
        for n in ("dbg_kT", "dbg_reads"):
            dbg[n] = nc.declare_dram_parameter(n, [128, NTOK], BF16, isOutput=True)
        dbg["dbg_state"] = nc.declare_dram_parameter("dbg_state", [128, BL * M], F32, isOutput=True)
        dbg["dbg_prob"] = nc.declare_dram_parameter("dbg_prob", [1, NTOK], F32, isOutput=True)
        dbg["dbg_wd9"] = nc.declare_dram_parameter("dbg_wd9", [T_PAD, 9, 400], BF16, isOutput=True)
        dbg["dbg_lg9"] = nc.declare_dram_parameter("dbg_lg9", [T_PAD, 9, 128], BF16, isOutput=True)
        dbg["dbg_la9"] = nc.declare_dram_parameter("dbg_la9", [T_PAD, 9, 128], BF16, isOutput=True)

    with tile.TileContext(nc) as tc:
        _emit(nc, tc, h, out_h, dbg, phases)
    nc.finalize()
    return nc


def _emit(nc, tc, h, out_h, dbg=None, phases=(1, 2, 3, 4)):
    from contextlib import ExitStack

    ctx = ExitStack()
    with ctx:
        # ---- pools ----
        persist = ctx.enter_context(tc.tile_pool(name="persist", bufs=1))
        dram = ctx.enter_context(tc.tile_pool(name="dram", bufs=1, space="DRAM"))

        # DRAM coefficient streams
        wd9 = dram.tile([T_PAD, 9, 400], BF16)
        lg9 = dram.tile([T_PAD, 9, 128], BF16)
        la9 = dram.tile([T_PAD, 9, 128], BF16)

        # persistent SBUF archives ([d, token] layouts, t-major tokens)
        k_T = persist.tile([128, NTOK], BF16)
        v_T = persist.tile([128, NTOK], BF16)
        e_T = persist.tile([128, NTOK], BF16)
        erecip_T = persist.tile([128, NTOK], F32)
        a_T = persist.tile([128, NTOK], BF16)
        reads_T = persist.tile([128, NTOK], BF16)
        f_T = persist.tile([128, NTOK], F32)

        # small persistent constants / weights
        ident = persist.tile([128, 128], F32)
        make_identity(nc, ident)
        ident_bf = persist.tile([128, 128], BF16)
        nc.vector.tensor_copy(out=ident_bf, in_=ident)
        ones128 = persist.tile([128, 128], BF16)
        nc.vector.memset(ones128, 1.0)
        ones400 = persist.tile([128, 400], BF16)
        nc.vector.memset(ones400, 1.0)
        zeros1200 = persist.tile([128, 1200], BF16)
        nc.vector.memset(zeros1200, 0.0)
        zeros400f = persist.tile([1, 400], F32)
        nc.vector.memset(zeros400f, 0.0)

        We_f32 = persist.tile([128, 128], F32)
        nc.sync.dma_start(out=We_f32, in_=h["We"][:, :])
        We_sb = persist.tile([128, 128], BF16)
        nc.vector.tensor_copy(out=We_sb, in_=We_f32)
        Wa_f32 = persist.tile([128, 128], F32)
        nc.sync.dma_start(out=Wa_f32, in_=h["Wa"][:, :])
        Wa_sb = persist.tile([128, 128], BF16)
        nc.vector.tensor_copy(out=Wa_sb, in_=Wa_f32)
        Wf_r32 = persist.tile([128, 128], F32)
        nc.sync.dma_start(out=Wf_r32, in_=h["Wf"][0:128, :])
        Wf_r = persist.tile([128, 128], BF16)
        nc.vector.tensor_copy(out=Wf_r, in_=Wf_r32)
        Wf_k32 = persist.tile([128, 128], F32)
        nc.sync.dma_start(out=Wf_k32, in_=h["Wf"][128:256, :])
        Wf_k = persist.tile([128, 128], BF16)
        nc.vector.tensor_copy(out=Wf_k, in_=Wf_k32)
        Wab_sb = persist.tile([128, 1], F32)
        nc.sync.dma_start(out=Wab_sb, in_=h["Wab"][:, :])
        Wd_sb = persist.tile([128, 1], F32)
        nc.sync.dma_start(out=Wd_sb, in_=h["Wd"][:, :])
        Mk_sb = persist.tile([50, 128], F32)
        nc.sync.dma_start(out=Mk_sb, in_=h["Mk"][:, :])
        Mv0_sb = persist.tile([50, 128], F32)
        nc.sync.dma_start(out=Mv0_sb, in_=h["Mv0"][:, :])

        def col(name, n=128):
            t = persist.tile([n, 1], F32, name=name)
            nc.sync.dma_start(out=t, in_=_ap(h[name[:-4]][:], 0, [[1, n], [1, 1]]))
            return t

        be_col = col("be_col")
        ba_col = col("ba_col")
        bf_col = col("bf_col")
        neg_be_col = persist.tile([128, 1], F32)
        nc.vector.tensor_scalar(out=neg_be_col, in0=be_col, scalar1=-1.0, scalar2=None, op0=OP.mult)

        # ---- stream-layout views ----
        wd9f = wd9.rearrange("t r c -> t (r c)")
        lg9f = lg9.rearrange("t r c -> t (r c)")
        la9f = la9.rearrange("t r c -> t (r c)")

        # ---- transpose Mv0 and Mk once; Mv0^T kept in SBUF for state init ----
        Mv0T_sb = persist.tile([128, 50], F32)
        MkT_sb = persist.tile([128, 50], BF16)
        with tc.tile_pool(name="init_ps", bufs=1, space="PSUM") as initp:
            mv0t = initp.tile([128, 50], F32)
            nc.tensor.transpose(mv0t, Mv0_sb, ident[0:50, 0:50])
            nc.any.tensor_copy(out=Mv0T_sb, in_=mv0t)
            mkt_ps = initp.tile([128, 50], F32)
            nc.tensor.transpose(mkt_ps, Mk_sb, ident[0:50, 0:50])
            nc.any.tensor_copy(out=MkT_sb, in_=mkt_ps)

        # =========== phases 1+2 interleaved ===========
        if 1 not in phases or 2 not in phases:
            return
        idxk_dram = dram.tile([NTOK], I32)
        idxv_dram = dram.tile([NTOK], I32)
        ek_bf = dram.tile([NUM_CONCEPT, 128], BF16)
        ev_bf = dram.tile([2 * NUM_CONCEPT, 128], BF16)
        with tc.tile_pool(name="ph1", bufs=1) as ph1, \
             tc.tile_pool(name="ph1t", bufs=4) as ph1t, \
             tc.tile_pool(name="ph1ps", bufs=2, space="PSUM") as ph1ps, \
             tc.tile_pool(name="ph2", bufs=3) as ph2, \
             tc.tile_pool(name="ph2ps", bufs=1, space="PSUM") as ph2ps:

            # ---- index prep FIRST so the gather queue starts early ----
            cseq = ph1.tile([8, T_PAD], I32)
            crse = ph1.tile([8, T_PAD], I32)
            nc.vector.memset(cseq, 0)
            nc.vector.memset(crse, 0)
            nc.sync.dma_start(out=cseq[:, 0:T], in_=h["concept_seq"][:, :])
            nc.scalar.dma_start(out=crse[:, 0:T], in_=h["correct_seq"][:, :])

            # x = concept + 1000*correct (exact in fp32, cast back to ints)
            cseq_f = ph1.tile([8, T_PAD], F32)
            nc.vector.tensor_copy(out=cseq_f, in_=cseq)
            crse_f = ph1.tile([8, T_PAD], F32)
            nc.vector.tensor_copy(out=crse_f, in_=crse)
            x_f = ph1.tile([8, T_PAD], F32)
            nc.vector.scalar_tensor_tensor(out=x_f, in0=crse_f, scalar=float(NUM_CONCEPT),
                                           in1=cseq_f, op0=OP.mult, op1=OP.add)
            x_i = ph1.tile([8, T_PAD], I32)
            nc.vector.tensor_copy(out=x_i, in_=x_f)

            # bounce through DRAM to rewrap indices token-major: chunk g's
            # 128 tokens land as column g of a [128, NCH] tile
            ckw = ph1.tile([128, NCH], I32)
            xvw = ph1.tile([128, NCH], I32)
            for srct, drt, dstt, eng in ((x_i, idxv_dram, xvw, nc.scalar),
                                         (cseq, idxk_dram, ckw, nc.sync)):
                eng.dma_start(out=_ap(drt[:], 0, [[T_PAD, 8], [1, T_PAD]]), in_=srct)
                eng.dma_start(out=dstt, in_=_ap(drt[:], 0, [[1, 128], [128, NCH]]))

            # bf16 copies of the embedding tables in DRAM (halves gather
            # bytes); value table first so v-gathers start earliest.
            # 512 rows per DMA: row r=(c*128+p) -> SBUF [p, c*128+d]
            for src_h, dst_t, nrows in ((h["embed_value"], ev_bf, 2 * NUM_CONCEPT),
                                        (h["embed_key"], ek_bf, NUM_CONCEPT)):
                for r0 in range(0, nrows, 512):
                    nch4 = min(4, (nrows - r0 + 127) // 128)
                    nlast = min(128, nrows - r0 - (nch4 - 1) * 128)
                    wide = (nch4 - 1) * 128 + nlast
                    tl = ph1t.tile([128, 512], F32, tag="tcv")
                    eng = (nc.sync, nc.scalar)[(r0 // 512) % 2]
                    if nlast == 128:
                        eng.dma_start(
                            out=tl[:, 0:nch4 * 128].rearrange("p (c d) -> p c d", c=nch4),
                            in_=_ap(src_h[:, :], r0 * 128,
                                    [[128, 128], [128 * 128, nch4], [1, 128]]))
                    else:
                        if nch4 > 1:
                            eng.dma_start(
                                out=tl[:, 0:(nch4 - 1) * 128].rearrange("p (c d) -> p c d", c=nch4 - 1),
                                in_=_ap(src_h[:, :], r0 * 128,
                                        [[128, 128], [128 * 128, nch4 - 1], [1, 128]]))
                        eng.dma_start(
                            out=tl[0:nlast, (nch4 - 1) * 128:nch4 * 128],
                            in_=_ap(src_h[:, :], (r0 + (nch4 - 1) * 128) * 128,
                                    [[128, nlast], [1, 128]]))
                    tb = ph1t.tile([128, 512], BF16, tag="tcb")
                    nc.vector.tensor_copy(out=tb, in_=tl)
                    if nlast == 128:
                        eng.dma_start(
                            out=_ap(dst_t[:, :], r0 * 128,
                                    [[128, 128], [128 * 128, nch4], [1, 128]]),
                            in_=tb[:, 0:nch4 * 128].rearrange("p (c d) -> p c d", c=nch4))
                    else:
                        if nch4 > 1:
                            eng.dma_start(
                                out=_ap(dst_t[:, :], r0 * 128,
                                        [[128, 128], [128 * 128, nch4 - 1], [1, 128]]),
                                in_=tb[:, 0:(nch4 - 1) * 128].rearrange("p (c d) -> p c d", c=nch4 - 1))
                        eng.dma_start(
                            out=_ap(dst_t[:, :], (r0 + (nch4 - 1) * 128) * 128,
                                    [[128, nlast], [1, 128]]),
                            in_=tb[0:nlast, (nch4 - 1) * 128:nch4 * 128])

            # ---- fill DRAM streams (sync/scalar only; gpsimd queue is for
            # gathers) ----
            fillqs = (nc.sync, nc.sync)
            fq = 0
            for r0 in range(0, T_PAD, 128):
                for c0 in range(0, 3600, 1200):
                    fillqs[fq % 2].dma_start(out=wd9f[r0:r0 + 128, c0:c0 + 1200], in_=zeros1200)
                    fq += 1
                fillqs[fq % 2].dma_start(out=wd9[r0:r0 + 128, 0, :], in_=ones400); fq += 1
                fillqs[fq % 2].dma_start(out=lg9[r0:r0 + 128, 0, :], in_=ones128); fq += 1
                # lg rows 1-8 written by e pass; zero them anyway for padded tail rows
                fillqs[fq % 2].dma_start(out=lg9f[r0:r0 + 128, 128:1152],
                                         in_=zeros1200[:, 0:1024]); fq += 1
                fillqs[fq % 2].dma_start(out=la9f[r0:r0 + 128, 0:1152],
                                         in_=zeros1200[:, 0:1152]); fq += 1

            def gather_chunk(g, table, idx_tile, dst, tag, f32=False):
                if f32:
                    rows = ph1t.tile([128, 128], F32, tag=tag + "3")
                    nc.gpsimd.indirect_dma_start(
                        out=rows, out_offset=None, in_=table,
                        in_offset=bass.IndirectOffsetOnAxis(ap=idx_tile[:, g:g + 1], axis=0))
                    tps = ph1ps.tile([128, 128], F32, tag="gt3", bufs=1)
                    nc.tensor.transpose(tps, rows, ident)
                    _evac(nc, g, dst[:, 128 * g:128 * (g + 1)], tps)
                    return
                rows = ph1t.tile([128, 128], BF16, tag=tag)
                nc.gpsimd.indirect_dma_start(
                    out=rows, out_offset=None, in_=table,
                    in_offset=bass.IndirectOffsetOnAxis(ap=idx_tile[:, g:g + 1], axis=0))
                tps = ph1ps.tile([128, 128], BF16, tag="gt")
                nc.tensor.transpose(tps, rows, ident_bf)
                _evac(nc, g, dst[:, 128 * g:128 * (g + 1)], tps)

            # ---- loop A: v-gathers interleaved with e/a slice compute ----
            for c in range(8):
                for g in range(4 * c, 4 * c + 4):
                    if g < 8:
                        gather_chunk(g, h["embed_value"][:, :], xvw, v_T, "rv", f32=True)
                    else:
                        gather_chunk(g, ev_bf[:, :], xvw, v_T, "rv")
                sl = slice(c * 512, (c + 1) * 512)
                elog = ph2ps.tile([128, 512], F32, tag="ealog", bufs=2)
                nc.tensor.matmul(elog, We_sb, v_T[:, sl], start=True, stop=True)
                nc.scalar.activation(out=e_T[:, sl], in_=elog, func=AF.Sigmoid, bias=be_col)
                etmp = ph2.tile([128, 512], F32, tag="etmp")
                nc.scalar.activation(out=etmp, in_=elog, func=AF.Exp,
                                     bias=neg_be_col, scale=-1.0)
                nc.vector.tensor_scalar(out=erecip_T[:, sl], in0=etmp, scalar1=1.0,
                                        scalar2=None, op0=OP.add)
                alog = ph2ps.tile([128, 512], F32, tag="ealog", bufs=2)
                nc.tensor.matmul(alog, Wa_sb, v_T[:, sl], start=True, stop=True)
                nc.scalar.activation(out=a_T[:, sl], in_=alog, func=AF.Tanh, bias=ba_col)

            # ---- loop B: k-gathers interleaved with w softmax + lg/la rows,
            # t0-major, b inner; per-chunk results land in staging tiles that
            # flush as 3 wide DMAs per t0 block (12 scatter DMAs total) ----
            stg = {}
            for i in range(NCH):
                t0i, b = i // 8, i % 8
                p = 4 * b + t0i
                if b == 0:
                    stg_w = ph2.tile([128, 400], BF16, tag="stgw", bufs=2, name="stg_w")
                    stg_e = ph2.tile([128, 1024], BF16, tag="stge", bufs=2, name="stg_e")
                    stg_a = ph2.tile([128, 1024], BF16, tag="stga", bufs=2, name="stg_a")
                    stg = {"w": stg_w, "e": stg_e, "a": stg_a}
                gather_chunk(p, ek_bf[:, :], ckw, k_T, "rk")
                sl = slice(p * 128, (p + 1) * 128)
                wlog = ph2ps.tile([128, 50], F32, tag="wlog", bufs=1)
                nc.tensor.matmul(wlog, k_T[:, sl], MkT_sb, start=True, stop=True)
                negmax = ph2.tile([128, 1], F32, tag="negmax")
                nc.vector.tensor_reduce(out=negmax, in_=wlog, axis=mybir.AxisListType.X,
                                        op=OP.max, negate=True)
                wexp = ph2.tile([128, 50], F32, tag="wexp")
                sumexp = ph2.tile([128, 1], F32, tag="sumexp")
                nc.scalar.activation(out=wexp, in_=wlog, func=AF.Exp, bias=negmax,
                                     accum_out=sumexp)
                rsum = ph2.tile([128, 1], F32, tag="rsum")
                nc.vector.reciprocal(out=rsum, in_=sumexp)
                nc.vector.tensor_scalar(out=stg["w"][:, b * 50:(b + 1) * 50], in0=wexp,
                                        scalar1=rsum, scalar2=None, op0=OP.mult)

                # e rows -> lg9 rows 1..8 (negated); a rows -> la9 rows 1..8
                ert = ph2ps.tile([128, 128], BF16, tag="eat", bufs=2)
                nc.tensor.transpose(ert, e_T[:, sl], ident_bf)
                if p % 2 == 0:
                    nc.vector.tensor_scalar(out=stg["e"][:, b * 128:(b + 1) * 128],
                                            in0=ert, scalar1=-1.0, scalar2=None,
                                            op0=OP.mult)
                else:
                    nc.scalar.mul(out=stg["e"][:, b * 128:(b + 1) * 128], in_=ert,
                                  mul=-1.0)
                art = ph2ps.tile([128, 128], BF16, tag="eat", bufs=2)
                nc.tensor.transpose(art, a_T[:, sl], ident_bf)
                if p % 2 == 0:
                    nc.scalar.copy(out=stg["a"][:, b * 128:(b + 1) * 128], in_=art)
                else:
                    nc.vector.tensor_copy(out=stg["a"][:, b * 128:(b + 1) * 128],
                                          in_=art)

                if b == 7:
                    t0 = 128 * t0i
                    nc.sync.dma_start(
                        out=_ap(wd9f[:], t0 * W9 + 400, [[W9, 128], [450, 8], [1, 50]]),
                        in_=stg["w"].rearrange("p (b c) -> p b c", b=8))
                    nc.scalar.dma_start(out=lg9f[t0:t0 + 128, 128:1152], in_=stg["e"])
                    nc.sync.dma_start(out=la9f[t0:t0 + 128, 128:1152], in_=stg["a"])

        # =========== phase 3: the 500-step recurrence ===========
        if 3 not in phases:
            return
        # v4: PE-accumulate chain (state in PSUM banks) + batched read
        # finalize. Per step t (state Mv_t in bank st[t%3]):
        #   PE   : G_t = lg9^T wd9 -> G-ring bank      (start/stop)
        #   ACT  : g_sb = copy(G_t)                    (PSUM -> SBUF)
        #   DVE  : S_t = reduce_m(prev) -> S ring      (off-chain)
        #   DVE  : new = prev * g_sb                   (chain)
        #   PE   : new += la9^T wd9                    (accum, chain)
        #   Pool : every RB steps, 3 batched TTs finalize reads:
        #          read_t = (S_t - S_{t+1} + a_t) * erecip_t
        # The accumulate relies on has_written=1 across each state bank,
        # arranged by a start=True zero-matmul before the loop.
        RB = 50
        state_fin = persist.tile([128, BL * M], F32)
        with tc.tile_pool(name="rec_ld", bufs=2) as rec_ld, \
             tc.tile_pool(name="rec_gw", bufs=6) as rec_gw, \
             tc.tile_pool(name="rec_stp", bufs=1) as rec_stp, \
             tc.tile_pool(name="rec_sm", bufs=2) as rec_sm, \
             tc.tile_pool(name="rec_st", bufs=1, space="PSUM") as rec_st, \
             tc.tile_pool(name="rec_ps", bufs=5, space="PSUM") as rec_ps:

            st = []
            for i in range(3):
                bank = rec_st.tile([128, BL * M], F32, name=f"stbank{i}")
                nc.tensor.matmul(bank, zeros1200[0:1, 0:128], zeros1200[0:1, 0:400],
                                 start=True, stop=True)
                st.append(bank)
            for b in range(BL):
                nc.scalar.copy(out=st[0][:, b * M:(b + 1) * M], in_=Mv0T_sb)

            # S ring, j-major: cols [j*BL, (j+1)*BL) hold S_{t0+j}
            S_ring = rec_stp.tile([128, BL * (RB + 1)], F32)
            ring_jb = S_ring.rearrange("p (j b) -> p j b", b=BL)
            a_tb = a_T.rearrange("p (b t) -> p t b", t=T_PAD)
            er_tb = erecip_T.rearrange("p (b t) -> p t b", t=T_PAD)
            rd_tb = reads_T.rearrange("p (b t) -> p t b", t=T_PAD)

            def finalize_reads(t0, nn):
                # reads for steps [t0, t0+nn) from S_ring slots 0..nn (Pool)
                d1 = rec_sm.tile([128, BL * RB], F32, tag="d1")
                d1v = d1.rearrange("p (j b) -> p j b", b=BL)
                nc.gpsimd.tensor_tensor(out=d1v[:, 0:nn, :], in0=ring_jb[:, 0:nn, :],
                                        in1=ring_jb[:, 1:nn + 1, :], op=OP.subtract)
                d2 = rec_sm.tile([128, BL * RB], F32, tag="d2")
                d2v = d2.rearrange("p (j b) -> p j b", b=BL)
                nc.gpsimd.tensor_tensor(out=d2v[:, 0:nn, :], in0=d1v[:, 0:nn, :],
                                        in1=a_tb[:, t0:t0 + nn, :], op=OP.add)
                nc.gpsimd.tensor_tensor(out=rd_tb[:, t0:t0 + nn, :],
                                        in0=d2v[:, 0:nn, :],
                                        in1=er_tb[:, t0:t0 + nn, :], op=OP.mult)
                # carry last S to slot 0 for the next block
                nc.gpsimd.tensor_copy(out=S_ring[:, 0:BL],
                                      in_=S_ring[:, nn * BL:(nn + 1) * BL])

            def reduce_S(t, bank):
                j = RB if (t % RB == 0 and t > 0) else t % RB
                nc.vector.tensor_reduce(
                    out=S_ring[:, j * BL:(j + 1) * BL],
                    in_=bank.rearrange("p (b m) -> p b m", b=BL),
                    axis=mybir.AxisListType.X, op=OP.add)

            SB = 8                       # sub-block: G matmuls run ahead
            for j0 in range(0, T, STEP_CHUNK):
                n = min(STEP_CHUNK, T - j0)
                wd_c = rec_ld.tile([9, STEP_CHUNK * 400], BF16, tag="wd_c")
                nc.sync.dma_start(out=wd_c[:, 0:n * 400],
                                  in_=_ap(wd9f[:], j0 * W9, [[400, 9], [W9, n], [1, 400]]))
                lg_c = rec_ld.tile([9, STEP_CHUNK * 128], BF16, tag="lg_c")
                nc.sync.dma_start(out=lg_c[:, 0:n * 128],
                                  in_=_ap(lg9f[:], j0 * L9, [[128, 9], [L9, n], [1, 128]]))
                la_c = rec_ld.tile([9, STEP_CHUNK * 128], BF16, tag="la_c")
                nc.sync.dma_start(out=la_c[:, 0:n * 128],
                                  in_=_ap(la9f[:], j0 * A9, [[128, 9], [A9, n], [1, 128]]))

                for s0 in range(0, n, SB):
                    gsbs = {}
                    for s in range(s0, min(s0 + SB, n)):
                        g_ps = rec_ps.tile([128, BL * M], F32, tag="g")
                        nc.tensor.matmul(g_ps, lg_c[:, s * 128:(s + 1) * 128],
                                         wd_c[:, s * 400:(s + 1) * 400],
                                         start=True, stop=True)
                        g_sb = rec_gw.tile([128, BL * M], F32, tag="g_sb")
                        nc.scalar.copy(out=g_sb, in_=g_ps)
                        gsbs[s] = g_sb
                    for s in range(s0, min(s0 + SB, n)):
                        t = j0 + s
                        prev, new = st[t % 3], st[(t + 1) % 3]
                        nc.vector.tensor_tensor(out=new, in0=prev, in1=gsbs[s],
                                                op=OP.mult)
                        nc.tensor.matmul(new, la_c[:, s * 128:(s + 1) * 128],
                                         wd_c[:, s * 400:(s + 1) * 400], start=False,
                                         stop=True, skip_group_check=True)
                        # S_t from the completed prev bank; AFTER the mult so
                        # the TR fills DVE slack while the PE accumulates WA
                        reduce_S(t, prev)
                        if t % RB == 0 and t > 0:
                            finalize_reads(t - RB, RB)
            # epilogue: S_500 -> slot 50, last fin block
            reduce_S(T, st[T % 3])
            finalize_reads(T - RB, RB)
            if dbg:
                nc.scalar.copy(out=state_fin, in_=st[T % 3])

        # =========== phase 4: output heads ===========
        if 4 not in phases:
            return
        # zero the padded t in [500, 512) columns of every batch row
        nc.vector.memset(
            reads_T.rearrange("p (b t) -> p b t", t=T_PAD)[:, :, T:T_PAD], 0.0)
        with tc.tile_pool(name="fin", bufs=2) as fin, \
             tc.tile_pool(name="finps", bufs=2, space="PSUM") as finps:
            for c in range(8):
                sl = slice(c * 512, (c + 1) * 512)
                f_ps = finps.tile([128, 512], F32, tag="f_ps")
                nc.tensor.matmul(f_ps, Wf_r, reads_T[:, sl], start=True, stop=False)
                nc.tensor.matmul(f_ps, Wf_k, k_T[:, sl], start=False, stop=True)
                nc.scalar.activation(out=f_T[:, sl], in_=f_ps, func=AF.Tanh, bias=bf_col)

            # heads: [2, 512] PSUM per chunk = {stu_logit; qd_logit}
            Wab0 = fin.tile([128, 2], F32, tag="wab0")
            nc.vector.memset(Wab0, 0.0)
            nc.sync.dma_start(out=Wab0[:, 0:1], in_=h["Wab"][:, :])
            W0d = fin.tile([128, 2], BF16, tag="w0d")
            nc.vector.memset(W0d, 0.0)
            nc.vector.tensor_copy(out=W0d[:, 1:2], in_=Wd_sb)
            comb = fin.tile([2, 1], F32, tag="comb")
            nc.vector.memset(comb, -1.0)
            nc.vector.memset(comb[0:1, :], 3.0)
            bias2 = fin.tile([2, 1], F32, tag="bias2")
            nc.sync.dma_start(out=bias2[0:1, :], in_=_ap(h["bab"][:], 0, [[1, 1], [1, 1]]))
            nc.sync.dma_start(out=bias2[1:2, :], in_=_ap(h["bd"][:], 0, [[1, 1], [1, 1]]))

            prob_row = fin.tile([1, NTOK], F32, tag="prob_row")
            for c in range(8):
                sl = slice(c * 512, (c + 1) * 512)
                hp = finps.tile([2, 512], F32, tag="hp")
                nc.tensor.matmul(hp, Wab0, f_T[:, sl], start=True, stop=False)
                nc.tensor.matmul(hp, W0d, k_T[:, sl], start=False, stop=True)
                ht = fin.tile([2, 512], F32, tag="ht")
                nc.scalar.activation(out=ht, in_=hp, func=AF.Tanh, bias=bias2)
                lg_ps = finps.tile([1, 512], F32, tag="lg_ps")
                nc.tensor.matmul(lg_ps, comb, ht, start=True, stop=True)
                nc.scalar.activation(out=prob_row[:, sl], in_=lg_ps, func=AF.Sigmoid)
            # prob_row[0, b*512+t] -> out[b, t], one strided DMA
            nc.sync.dma_start(
                out=_ap(out_h[:, :], 0, [[1, 1], [T, BL], [1, T]]),
                in_=_ap(prob_row[0:1, :], 0, [[1, 1], [T_PAD, BL], [1, T]]))
            if dbg:
                for name, tile_ in (("dbg_kT", k_T), ("dbg_eT", e_T), ("dbg_aT", a_T),
                                    ("dbg_erecip", erecip_T), ("dbg_reads", reads_T),
                                    ("dbg_fT", f_T), ("dbg_state", state_fin),
                                    ("dbg_prob", prob_row)):
                    nc.sync.dma_start(out=dbg[name][:, :], in_=tile_)
                nc.sync.dma_start(out=dbg["dbg_wd9"][:, :, :], in_=wd9)
                nc.sync.dma_start(out=dbg["dbg_lg9"][:, :, :], in_=lg9)
                nc.sync.dma_start(out=dbg["dbg_la9"][:, :, :], in_=la9)


_NC = None
LAST_RESULT = None


def _get_nc():
    global _NC
    if _NC is None:
        _NC = build_program()
    return _NC


def kernel(**inputs):
    global LAST_RESULT
    from concourse.bass_utils import run_bass_kernel_spmd

    nc = _get_nc()
    names = ["concept_seq", "correct_seq", "embed_key", "embed_value", "Mk", "Mv0",
             "We", "be", "Wa", "ba", "Wf", "bf", "Wab", "bab", "Wd", "bd"]
    full = {k: np.ascontiguousarray(np.asarray(inputs[k])) for k in names}
    in_maps = []
    for i in range(NCORES):
        m = dict(full)
        m["concept_seq"] = np.ascontiguousarray(full["concept_seq"][i * BL:(i + 1) * BL])
        m["correct_seq"] = np.ascontiguousarray(full["correct_seq"][i * BL:(i + 1) * BL])
        in_maps.append(m)
    res = run_bass_kernel_spmd(nc, in_maps, core_ids=list(range(NCORES)))
    LAST_RESULT = res
    return np.concatenate([res.results[i]["out"] for i in range(NCORES)], axis=0)


if __name__ == "__main__":
    nc = build_program()
    print("build ok:", len(nc.m.functions[0].instructions) if hasattr(nc.m.functions[0], "instructions") else "n/a")

